# revision 62
# baseline (speedup 1.0000x reference)
"""Trainium2 Bass kernel for per-head bilinear graph attention.

Reference computation (B=4, N=2048, IN=256, H=8, ATN=32):
    xt     = einsum('bni,hio->bhno', x, W) + b          # [B,H,N,32]
    xC     = einsum('bhno,hpo->bhnp', xt, C)            # [B,H,N,32]
    scores = einsum('bhnp,bhmp->bhnm', xC, xt)          # [B,H,N,N]
    alpha  = tanh(scores * adj[:,None])                 # [B,H,N,N]
    heads  = einsum('bhnm,bhmo->bhno', alpha, xt)       # [B,H,N,32]
    out    = concat heads on feature dim                # [B,N,256]

Sharding: 8 cores = 4 batches x 2 head-groups (4 heads each). Fully
data-parallel, no collectives. Each core computes out[b, :, hg*128:(hg+1)*128]
transposed ([128, 2048]); the host transposes back and concatenates.

Device-side layout is fully transposed ("T" = [feature/m, n]):
    xtT  [128(4h x 32o), 2048n]   stacked per-head xt^T (bias included)
    xCT  [128(4h x 32p), 2048n]   stacked per-head xC^T
    sT   [128m, n]     = scores[n, m]   (psum, per m-chunk per head)
    z    = sT * adjT   (adjT host-pretransposed so it is [m, n])
    alphaT = tanh(z)
    outT [128(4h x 32o), 2048n] accumulated in psum over 16 m-chunks

Engine budget per core: the kernel sits at the DVE+ACT joint capacity
floor (TRN2 matmul psum output is fp32-only, so every score element must
exit PSUM through a 1x fp32 DVE/ACT op; tanh is ACT-only at 0.83ns/col).
The LP optimum routes 14 of the 128 multiply units through an ACT cast +
GPSIMD multiply and fuses the rest with the psum exit on DVE: ACT ~=
137.8us, DVE ~= 135.5us busy; measured exec 167.6-170.6us at nominal
clock, median ~168.8 (vs 179.5 for the paired-tanh v1 at the same
clock; runs draw a +-20% clock lottery, compare via exec / TANH-busy).
Remaining overhead is ~6.7us fixed NEFF launch, ~8us serial prologue
chain (DMA -> proj -> xC -> scores, latency-tight at each hop), ~5us
tail, ~12us three-engine phase jitter. The scheduling rules that
matter, each worth us-level regressions when violated:
 - per-mc z/alpha tiles [P,4,1024] with tanh pipelined ONE m-chunk
   behind the multiplies (tanh(mc-1) after mc's scores+mults emission),
   deep rotations (z 7, alpha 6, adj 8) so phase jitter never couples
   the engines.
 - heads(mc-2) matmuls interleave BETWEEN the two scores q-groups: a
   contiguous 8-matmul heads burst drains the 3-slot scores window and
   starves DVE ~1.2us per 2 mc; parking heads in front of scores
   head-of-line blocks PE's 4-deep wait queue on ACT's tanh.
 - pool casts are emitted AFTER tanh(mc-1) so ACT never queues a cast
   whose PE slot isn't written yet ahead of a ready tanh; the GPSIMD
   multiply still has ~5us of margin before tanh(mc) needs its slice.
 - no zero-seed matmuls: has_written clears are per-partition-slice on
   this HW (the v1 diagonal xCT matmuls with start=True already relied
   on that), so heads/proj groups use start=(first k-chunk).
 - prologue: a no-dep dummy ACT op hoists the 1.3us activation-table
   load into the DMA window (it otherwise lands between proj0 and the
   first xtT copy); 13 real-sized (K=128, 512-col) dummy matmuls bank
   PE busy time so the mid->full p-state ramp happens inside the DMA
   window instead of stretching the proj/xC/scores chain 2x (tiny K=1
   dummies do NOT trigger the ramp). xT is loaded into per-chunk tiles
   so proj0 depends only on its own DMA; sync queue order xT0, W, bias,
   then adj (bias on the gpsimd SWDGE queue queues behind the 384KB xT
   bulk transfers and gates the first xtT copy ~2us). Only proj/xC
   chunk 0 gates mc0's q0 scores; chunk 1 lands between mc0's q-blocks,
   chunks 2,3 + the xt4 transpose groups ride pool-routed m-chunks
   where DVE has slack.
 - tail: mc15's tanh+heads+output copy are emitted per-q, and the very
   last 512-col block splits its copy across ACT+DVE halves with two
   chasing DMAs.

Key facts baked into this design (from HW traces + the CoreSim cost
model source):
 - DVE tensor_tensor 2x mode needs ALL operands 2-byte; fp32 psum scores
   therefore pin the mask-multiply at 1x. TRN2 matmul cannot write bf16
   psum (is_transpose=True really is a different PE datapath - measured
   garbage), and DMA has no PSUM port, so there is no cheaper exit.
 - GPSIMD cannot access PSUM; its multiplies need an ACT cast first
   (1.0us/unit ACT buys 1.15us/unit off DVE - profitable only while
   ACT's tanh load leaves it slack).
 - K=32 scores matmuls are packed pairwise into PE row-groups
   (tile_position) with [128,2,512] psum tiles; outT uses 4-way
   col-group packing. PSUM: 3x2-bank scores slots + 2x1-bank output
   accumulators = 8 banks.
"""

import sys
import types

import numpy as np
import ml_dtypes

BF16_NP = ml_dtypes.bfloat16


def _ensure_axon_ntff_hook():
    """Provide antenv.axon_hooks if the image lacks it, so
    run_bass_kernel_spmd(trace=True) can capture NTFF profiles instead of
    crashing on the import. No-op when the real module exists."""
    try:
        import antenv.axon_hooks  # noqa: F401

        return
    except ImportError:
        pass
    mod = types.ModuleType("antenv.axon_hooks")
    _state = {"hook": None}
    mod.set_axon_ntff_profile_hook = lambda h: _state.__setitem__("hook", h)
    mod.get_axon_ntff_profile_hook = lambda: _state["hook"]
    sys.modules["antenv.axon_hooks"] = mod
    try:
        import antenv

        antenv.axon_hooks = mod
    except ImportError:
        pass
    try:
        from trn_agent_boot.trn_boot import _ntff_profile_via_ctypes

        mod.set_axon_ntff_profile_hook(
            _ntff_profile_via_ctypes("/opt/axon/libaxon_pjrt.so")
        )
    except Exception:
        pass


_ensure_axon_ntff_hook()

from concourse import bacc, mybir, tile
import concourse.bass as bass
from concourse.bass_utils import run_bass_kernel_spmd

F32 = mybir.dt.float32
BF16 = mybir.dt.bfloat16
AF = mybir.ActivationFunctionType
ALU = mybir.AluOpType

P = 128
B, N, IN_DIM, H, ATN = 4, 2048, 256, 8, 32
NH = 4                # heads per core
NCORES = 8
MC = N // P           # 16 m-chunks
IC = IN_DIM // P      # 2 contraction chunks for the input projection

# m-chunks whose (hp=0, q=0) multiply slice is routed ACT-cast -> GPSIMD
# (per n-half). 7 per half is the measured optimum (g=12/16/18 and the
# even-mc set all cost 1-8us). The cast is emitted right after
# tanh(mc-1): the Pool multiply still has ~5us of margin before tanh(mc)
# needs its z slice, and ACT never queues a cast whose PE slot isn't
# ready ahead of a tanh that is.
POOL_MCS = (1, 3, 5, 7, 9, 11, 13)
POOL_MCS_NH = ((0, 3, 5, 7, 9, 11, 13), POOL_MCS)

_CACHE = {}


def build_graph():
    nc = bacc.Bacc("TRN2", target_bir_lowering=False, debug=False)

    xT_d = nc.dram_tensor("xT", [IN_DIM, N], BF16, kind="ExternalInput")
    id_d = nc.dram_tensor("ident", [P, P], BF16, kind="ExternalInput")
    adjT_d = nc.dram_tensor("adjT", [N, N], BF16, kind="ExternalInput")
    # weights: [P, IC*NH*ATN] W-part ++ [P, ATN] C^T-part, one fast DMA
    W_d = nc.dram_tensor("Wt", [P, IC * NH * ATN + ATN], BF16, kind="ExternalInput")
    b_d = nc.dram_tensor("bias", [P, 1], F32, kind="ExternalInput")
    out_d = nc.dram_tensor("out", [P, N], BF16, kind="ExternalOutput")

    with tile.TileContext(nc) as tc:
        with (
            tc.tile_pool(name="const", bufs=1) as cp,
            tc.tile_pool(name="adj", bufs=8) as adjp,
            tc.tile_pool(name="z", bufs=7) as zp,
            tc.tile_pool(name="alpha", bufs=6) as alp,
            tc.tile_pool(name="cast", bufs=3) as scp,
            tc.tile_pool(name="ps_o", bufs=2, space="PSUM") as ps_o,
            tc.tile_pool(name="ps_s", bufs=3, space="PSUM") as ps_s,
        ):
            # The ACT activation-table load costs 1.3us and the framework
            # inserts it before ACT's first instruction: give ACT a no-dep
            # dummy op up front so the load overlaps the input DMA window
            # instead of blocking the first xtT copy. (tanh/identity/copy
            # share one table set, so it loads exactly once.)
            warm = cp.tile([1, 8], BF16)
            nc.gpsimd.memset(warm[:], 0.0)
            nc.scalar.activation(warm[:], warm[:], AF.Tanh)
            # PE p-state warm-up: the PE clock ramps mid -> full only after
            # several us of accumulated busy time, and the serial prologue
            # chain (proj -> xC -> scores) otherwise runs ~2x slow (512-col
            # matmuls measure 0.58us vs 0.26us steady-state). Real-sized
            # (K=128, 512-col) dummy matmuls during the DMA window bank
            # that busy time before proj0 arrives; tiny K=1 dummies do NOT
            # trigger the ramp.
            warmbig = cp.tile([P, 512], BF16)
            nc.gpsimd.memset(warmbig[:], 0.0)
            wp = ps_s.tile([8, 512], F32, tag="s", name="warm_ps")
            for _ in range(13):
                nc.tensor.matmul(
                    wp[:], warmbig[:, :8], warmbig[:], start=True, stop=True,
                    skip_group_check=True,
                )

            # xT chunk0 heads the serial critical chain (proj -> xC ->
            # scores -> mult -> tanh): issue it FIRST on the sync queue,
            # split by contraction half so proj's c0 matmuls can start
            # ~0.4us before c1 lands. DMA issues serialize at ~0.65us each
            # per sequencer; bias/ident ride the gpsimd SWDGE queue so adj0
            # gets the third sync slot.
            # per-chunk xT tiles: proj(nq) depends only on its own chunk's
            # DMA, so no false tile-level coupling to the later bulk loads
            xT_sb = [
                cp.tile([P, IC, 512], BF16, name=f"xT_{nq}")
                for nq in range(N // 512)
            ]
            xT_src = xT_d[:].rearrange("(c p) n -> p c n", p=P)
            nc.sync.dma_start(xT_sb[0][:], xT_src[:, :, bass.ts(0, 512)])
            Wall_sb = cp.tile([P, IC * NH * ATN + ATN], BF16)
            nc.sync.dma_start(Wall_sb[:], W_d[:])
            # bias rides sync slot 3: on the gpsimd SWDGE queue it would
            # queue behind three 384KB xT bulk transfers and gate the first
            # xtT copy by ~2us
            b_sb = cp.tile([P, 1], F32)
            nc.sync.dma_start(b_sb[:], b_d[:])
            for nq in range(1, N // 512):
                nc.gpsimd.dma_start(
                    xT_sb[nq][:], xT_src[:, :, bass.ts(nq, 512)]
                )
            ident = cp.tile([P, P], BF16)
            nc.gpsimd.dma_start(ident[:], id_d[:])
            W_sb = Wall_sb[:, : IC * NH * ATN].rearrange(
                "p (c h o) -> p c h o", c=IC, h=NH
            )
            CT_sb = Wall_sb[:, IC * NH * ATN :]

            xtT = cp.tile([P, N], BF16)
            xCT = cp.tile([P, N], BF16)
            xt4 = cp.tile([P, MC, P], BF16)
            out_sb = cp.tile([P, N], BF16)

            # --- prologue pieces ---
            def emit_xtT(nq):
                # xtT[32h+o, n] = sum_i W[h,i,o] x[n,i] + b[h,o].
                # c-outer/h-inner so the 4 col-groups run concurrently in
                # the PE array; start=True on the first c-chunk per group
                # (has_written clear is per-partition-slice). The bias
                # rides on the ACT copy out of psum.
                pt = ps_s.tile([P, 1024], F32, tag="s", name=f"pj_{nq}")
                for c in range(IC):
                    for h in range(NH):
                        nc.tensor.matmul(
                            pt[bass.ts(h, ATN), :512],
                            W_sb[:, c, h, :],
                            xT_sb[nq][:, c, :],
                            start=(c == 0),
                            stop=(c == IC - 1),
                            tile_position=(0, h * ATN),
                            skip_group_check=True,
                        )
                nc.scalar.activation(
                    xtT[:, bass.ts(nq, 512)], pt[:, :512], AF.Identity, bias=b_sb[:]
                )

            def emit_xCT(nq, on_act=True):
                # xCT[32h+p, n] = sum_o C[h,p,o] xt[n,o]; diagonal 32x32
                # tiles run concurrently in distinct row+col groups.
                pt = ps_s.tile([P, 1024], F32, tag="s", name=f"xc_{nq}")
                for h in range(NH):
                    nc.tensor.matmul(
                        pt[bass.ts(h, ATN), :512],
                        CT_sb[bass.ts(h, ATN), :],
                        xtT[bass.ts(h, ATN), bass.ts(nq, 512)],
                        start=True,
                        stop=True,
                        tile_position=(h * ATN, h * ATN),
                        skip_group_check=True,
                    )
                if on_act:
                    nc.scalar.copy(xCT[:, bass.ts(nq, 512)], pt[:, :512])
                else:
                    nc.vector.tensor_copy(xCT[:, bass.ts(nq, 512)], pt[:, :512])

            def emit_xt4(g):
                # xt4[m_local, mc, f] = xt[mc*128+m_local, f]: PE transposes
                # of xtT, 4 m-chunks per psum tile (cycled through a ps_s
                # slot). Copies on DVE (bf16 psum -> bf16 sbuf, 2x path).
                pt = ps_s.tile([P, 4, P], BF16, tag="s", name=f"tr_{g}")
                for k in range(4):
                    nc.tensor.transpose(
                        pt[:, k, :], xtT[:, bass.ts(4 * g + k, P)], ident[:]
                    )
                nc.vector.tensor_copy(xt4[:, bass.ds(4 * g, 4), :], pt[:])

            # Only projection chunk 0 gates mc0's q0 scores; chunk 1 is
            # emitted between mc0's q-blocks (PROLOG_MID), and chunks 2,3
            # plus the transpose groups are spread through nh0's early
            # m-chunks where PE has slack. heads(mc) first fires at
            # iteration mc+2, so transpose group g is due by mc 4g.
            emit_xtT(0)
            emit_xCT(0)

            def _mid0():
                emit_xtT(1)
                emit_xCT(1)

            # Deferred prologue work rides the pool-routed m-chunks (DVE
            # does one less multiply there, so its copy slots in free);
            # transpose group g is due by iteration 4g+2, proj/xC chunk
            # k by the first scores m-chunk that reads xtT chunk k.
            PROLOG_MID = {0: _mid0}
            PROLOG_AT = {0: lambda: emit_xt4(0),
                         1: lambda: emit_xtT(2),
                         2: lambda: emit_xCT(2, on_act=False),
                         3: lambda: emit_xt4(1),
                         5: lambda: emit_xtT(3),
                         7: lambda: emit_xCT(3, on_act=False),
                         9: lambda: emit_xt4(2),
                         11: lambda: emit_xt4(3)}

            # --- main loop: n-half outer, m-chunks inner, tanh lagging the
            # multiplies by one m-chunk ---
            NHALF = N // 1024
            for nh in range(NHALF):
                # Two 1-bank accumulators; the q0 output copy fires as soon
                # as q0's last matmul stops, overlapping q1's.
                po_q = [
                    ps_o.tile([P, 512], F32, tag="po", name=f"po_{nh}_{q}")
                    for q in range(2)
                ]

                def emit_heads(mc, alpha, q, nh=nh):
                    for h in range(NH):
                        nc.tensor.matmul(
                            po_q[q][bass.ts(h, ATN), :],
                            xt4[:, mc, bass.ts(h, ATN)],
                            alpha[:, h, bass.ts(q, 512)],
                            start=(mc == 0),
                            stop=(mc == MC - 1),
                            tile_position=(0, h * ATN),
                            skip_group_check=True,
                        )

                zbuf = [None, None]
                abuf = [None, None]
                pool_defer = []

                def emit_tanh(mc, nh=nh):
                    alpha = alp.tile([P, NH, 1024], BF16, tag="alpha",
                                     name=f"al_{nh}_{mc}")
                    abuf[mc % 2] = alpha
                    nc.scalar.activation(alpha[:], zbuf[mc % 2][:], AF.Tanh)

                for mc in range(MC):
                    adjt = adjp.tile([P, 1024], BF16, tag="adj")
                    nc.sync.dma_start(
                        adjt[:], adjT_d[bass.ts(mc, P), bass.ds(nh * 1024, 1024)]
                    )
                    zb = zp.tile([P, NH, 1024], BF16, tag="z",
                                 name=f"zb_{nh}_{mc}")
                    zbuf[mc % 2] = zb
                    pool_mc = mc in POOL_MCS_NH[nh]
                    # heads(mc-2) interleaved BETWEEN scores groups: its tanh
                    # landed a full m-chunk ago so it never parks waiting
                    # instrs in front of the scores stream (the 4-deep PE
                    # wait queue would head-of-line block DVE's multiply
                    # feed), and splitting the 8-matmul heads burst in two
                    # keeps the 3-slot scores window from draining.
                    # NOTE: the pool-routed slice must be produced FIRST.
                    # Producing it LAST serializes the kernel on an
                    # ACT->GPSIMD->ACT chain (+2.2us per pool mc, measured
                    # 194us); even SECOND measures ~+20us - any
                    # displacement perturbs the 3-slot ps_s rotation more
                    # than the ~4us cast slot-hold costs.
                    qhps = [(q, hp) for q in range(2) for hp in range(NH // 2)]
                    q_seen = set()
                    for q, hp in qhps:
                        if q not in q_seen:
                            q_seen.add(q)
                            if nh == 0 and q == 1 and mc in PROLOG_MID:
                                PROLOG_MID[mc]()
                            if mc >= 2:
                                emit_heads(mc - 2, abuf[mc % 2], q)
                        s2 = ps_s.tile([P, 2, 512], F32, tag="s")
                        for j in range(2):
                            h = 2 * hp + j
                            nc.tensor.matmul(
                                s2[:, j, :],
                                xtT[bass.ts(h, ATN), bass.ts(mc, P)],
                                xCT[
                                    bass.ts(h, ATN),
                                    bass.ds(nh * 1024 + q * 512, 512),
                                ],
                                start=True,
                                stop=True,
                                tile_position=(h * ATN, 0),
                                skip_group_check=True,
                            )
                        zsl = zb[:, bass.ds(2 * hp, 2), bass.ts(q, 512)]
                        adj_b = adjt[:, None, bass.ts(q, 512)].to_broadcast(
                            (P, 2, 512)
                        )
                        if pool_mc and q == 0 and hp == 0:
                            # deferred: ACT casts psum->bf16 after
                            # tanh(mc-1); the otherwise-idle GPSIMD
                            # engine does the multiply (it has no PSUM
                            # port, hence the cast).
                            pool_defer.append((s2, zsl, adj_b))
                        else:
                            nc.vector.tensor_tensor(zsl, s2[:], adj_b, ALU.mult)
                    if nh == 0 and mc in PROLOG_AT:
                        PROLOG_AT[mc]()
                    if mc >= 1:
                        emit_tanh(mc - 1)
                    for (s2, zsl, adj_b) in pool_defer:
                        sc = scp.tile([P, 2, 512], BF16, tag="cast")
                        nc.scalar.copy(sc[:], s2[:])
                        nc.gpsimd.tensor_tensor(zsl, sc[:], adj_b, ALU.mult)
                    pool_defer.clear()
                # tail: heads(14), then per-q tanh(15) + heads(15) + output
                # copy + DMA so the last 512-col DMA chases the last matmul
                for q in range(2):
                    emit_heads(MC - 2, abuf[(MC - 2) % 2], q)
                alpha = alp.tile([P, NH, 1024], BF16, tag="alpha",
                                 name=f"al_{nh}_{MC - 1}")
                for q in range(2):
                    nc.scalar.activation(
                        alpha[:, :, bass.ts(q, 512)],
                        zbuf[(MC - 1) % 2][:, :, bass.ts(q, 512)],
                        AF.Tanh,
                    )
                    emit_heads(MC - 1, alpha, q)
                    if nh == NHALF - 1 and q == 1:
                        # very last output block: split the copy across
                        # ACT+DVE halves with two chasing DMAs to shorten
                        # the serial tail
                        for half, eng in ((0, nc.scalar.copy),
                                          (1, nc.vector.tensor_copy)):
                            off = nh * 1024 + q * 512 + half * 256
                            eng(out_sb[:, bass.ds(off, 256)],
                                po_q[q][:, bass.ts(half, 256)])
                            nc.sync.dma_start(
                                out_d[:, bass.ds(off, 256)],
                                out_sb[:, bass.ds(off, 256)],
                            )
                    else:
                        nc.vector.tensor_copy(
                            out_sb[:, bass.ds(nh * 1024 + q * 512, 512)],
                            po_q[q][:],
                        )
                        nc.sync.dma_start(
                            out_d[:, bass.ds(nh * 1024 + q * 512, 512)],
                            out_sb[:, bass.ds(nh * 1024 + q * 512, 512)],
                        )

    nc.compile()
    return nc


def _get_graph():
    if "nc" not in _CACHE:
        _CACHE["nc"] = build_graph()
    return _CACHE["nc"]


def make_in_maps(x, adj, W, b, C):
    in_maps = []
    for core in range(NCORES):
        bb = core // 2
        hg = core % 2
        hs = slice(hg * NH, (hg + 1) * NH)
        Wt = (
            W[hs]
            .reshape(NH, IC, P, ATN)
            .transpose(2, 1, 0, 3)
            .reshape(P, IC * NH * ATN)
        )
        CTt = C[hs].transpose(0, 2, 1).reshape(NH * ATN, ATN)
        in_maps.append(
            {
                "xT": np.ascontiguousarray(x[bb].T).astype(BF16_NP),
                "ident": np.eye(P, dtype=np.float32).astype(BF16_NP),
                "adjT": np.ascontiguousarray(adj[bb].T).astype(BF16_NP),
                "Wt": np.ascontiguousarray(
                    np.concatenate([Wt, CTt], axis=1)
                ).astype(BF16_NP),
                "bias": np.ascontiguousarray(b[hs].reshape(P, 1)),
            }
        )
    return in_maps


LAST_RESULT = None


def kernel(x, adj, W, b, C):
    global LAST_RESULT
    x = np.asarray(x, dtype=np.float32)
    adj = np.asarray(adj, dtype=np.float32)
    W = np.asarray(W, dtype=np.float32)
    b = np.asarray(b, dtype=np.float32)
    C = np.asarray(C, dtype=np.float32)

    nc = _get_graph()
    in_maps = make_in_maps(x, adj, W, b, C)
    res = run_bass_kernel_spmd(nc, in_maps, core_ids=list(range(NCORES)))
    LAST_RESULT = res

    out = np.empty((B, N, H * ATN), dtype=np.float32)
    for core in range(NCORES):
        bb = core // 2
        hg = core % 2
        out[bb, :, hg * P : (hg + 1) * P] = (
            res.results[core]["out"].astype(np.float32).T
        )
    return out


# revision 63
# speedup vs baseline: 1.0025x; 1.0025x over previous
"""Trainium2 Bass kernel for per-head bilinear graph attention.

Reference computation (B=4, N=2048, IN=256, H=8, ATN=32):
    xt     = einsum('bni,hio->bhno', x, W) + b          # [B,H,N,32]
    xC     = einsum('bhno,hpo->bhnp', xt, C)            # [B,H,N,32]
    scores = einsum('bhnp,bhmp->bhnm', xC, xt)          # [B,H,N,N]
    alpha  = tanh(scores * adj[:,None])                 # [B,H,N,N]
    heads  = einsum('bhnm,bhmo->bhno', alpha, xt)       # [B,H,N,32]
    out    = concat heads on feature dim                # [B,N,256]

Sharding: 8 cores = 4 batches x 2 head-groups (4 heads each). Fully
data-parallel, no collectives. Each core computes out[b, :, hg*128:(hg+1)*128]
transposed ([128, 2048]); the host transposes back and concatenates.

Device-side layout is fully transposed ("T" = [feature/m, n]):
    xtT  [128(4h x 32o), 2048n]   stacked per-head xt^T (bias included)
    xCT  [128(4h x 32p), 2048n]   stacked per-head xC^T
    sT   [128m, n]     = scores[n, m]   (psum, per m-chunk per head)
    z    = sT * adjT   (adjT host-pretransposed so it is [m, n])
    alphaT = tanh(z)
    outT [128(4h x 32o), 2048n] accumulated in psum over 16 m-chunks

Engine budget per core: the kernel sits at the DVE+ACT joint capacity
floor (TRN2 matmul psum output is fp32-only, so every score element must
exit PSUM through a 1x fp32 DVE/ACT op; tanh is ACT-only at 0.83ns/col).
The LP optimum routes 14 of the 128 multiply units through an ACT cast +
GPSIMD multiply and fuses the rest with the psum exit on DVE: ACT ~=
137.8us, DVE ~= 135.5us busy; measured exec 167.6-170.6us at nominal
clock, median ~168.8 (vs 179.5 for the paired-tanh v1 at the same
clock; runs draw a +-20% clock lottery, compare via exec / TANH-busy).
Remaining overhead is ~6.7us fixed NEFF launch, ~8us serial prologue
chain (DMA -> proj -> xC -> scores, latency-tight at each hop), ~5us
tail, ~12us three-engine phase jitter. The scheduling rules that
matter, each worth us-level regressions when violated:
 - per-mc z/alpha tiles [P,4,1024] with tanh pipelined ONE m-chunk
   behind the multiplies (tanh(mc-1) after mc's scores+mults emission),
   deep rotations (z 7, alpha 6, adj 8) so phase jitter never couples
   the engines.
 - heads(mc-2) matmuls interleave BETWEEN the two scores q-groups: a
   contiguous 8-matmul heads burst drains the 3-slot scores window and
   starves DVE ~1.2us per 2 mc; parking heads in front of scores
   head-of-line blocks PE's 4-deep wait queue on ACT's tanh.
 - pool casts are emitted AFTER tanh(mc-1) so ACT never queues a cast
   whose PE slot isn't written yet ahead of a ready tanh; the GPSIMD
   multiply still has ~5us of margin before tanh(mc) needs its slice.
 - no zero-seed matmuls: has_written clears are per-partition-slice on
   this HW (the v1 diagonal xCT matmuls with start=True already relied
   on that), so heads/proj groups use start=(first k-chunk).
 - prologue: a no-dep dummy ACT op hoists the 1.3us activation-table
   load into the DMA window (it otherwise lands between proj0 and the
   first xtT copy); 13 real-sized (K=128, 512-col) dummy matmuls bank
   PE busy time so the mid->full p-state ramp happens inside the DMA
   window instead of stretching the proj/xC/scores chain 2x (tiny K=1
   dummies do NOT trigger the ramp). xT is loaded into per-chunk tiles
   so proj0 depends only on its own DMA; sync queue order xT0, W, bias,
   then adj (bias on the gpsimd SWDGE queue queues behind the 384KB xT
   bulk transfers and gates the first xtT copy ~2us). Only proj/xC
   chunk 0 gates mc0's q0 scores; chunk 1 lands between mc0's q-blocks,
   chunks 2,3 + the xt4 transpose groups ride pool-routed m-chunks
   where DVE has slack.
 - tail: mc15's tanh+heads+output copy are emitted per-q, and the very
   last 512-col block splits its copy across ACT+DVE halves with two
   chasing DMAs.

Key facts baked into this design (from HW traces + the CoreSim cost
model source):
 - DVE tensor_tensor 2x mode needs ALL operands 2-byte; fp32 psum scores
   therefore pin the mask-multiply at 1x. TRN2 matmul cannot write bf16
   psum (is_transpose=True really is a different PE datapath - measured
   garbage), and DMA has no PSUM port, so there is no cheaper exit.
 - GPSIMD cannot access PSUM; its multiplies need an ACT cast first
   (1.0us/unit ACT buys 1.15us/unit off DVE - profitable only while
   ACT's tanh load leaves it slack).
 - K=32 scores matmuls are packed pairwise into PE row-groups
   (tile_position) with [128,2,512] psum tiles; outT uses 4-way
   col-group packing. PSUM: 3x2-bank scores slots + 2x1-bank output
   accumulators = 8 banks.
"""

import sys
import types

import numpy as np
import ml_dtypes

BF16_NP = ml_dtypes.bfloat16


def _ensure_axon_ntff_hook():
    """Provide antenv.axon_hooks if the image lacks it, so
    run_bass_kernel_spmd(trace=True) can capture NTFF profiles instead of
    crashing on the import. No-op when the real module exists."""
    try:
        import antenv.axon_hooks  # noqa: F401

        return
    except ImportError:
        pass
    mod = types.ModuleType("antenv.axon_hooks")
    _state = {"hook": None}
    mod.set_axon_ntff_profile_hook = lambda h: _state.__setitem__("hook", h)
    mod.get_axon_ntff_profile_hook = lambda: _state["hook"]
    sys.modules["antenv.axon_hooks"] = mod
    try:
        import antenv

        antenv.axon_hooks = mod
    except ImportError:
        pass
    try:
        from trn_agent_boot.trn_boot import _ntff_profile_via_ctypes

        mod.set_axon_ntff_profile_hook(
            _ntff_profile_via_ctypes("/opt/axon/libaxon_pjrt.so")
        )
    except Exception:
        pass


_ensure_axon_ntff_hook()

from concourse import bacc, mybir, tile
import concourse.bass as bass
from concourse.bass_utils import run_bass_kernel_spmd

F32 = mybir.dt.float32
BF16 = mybir.dt.bfloat16
AF = mybir.ActivationFunctionType
ALU = mybir.AluOpType

P = 128
B, N, IN_DIM, H, ATN = 4, 2048, 256, 8, 32
NH = 4                # heads per core
NCORES = 8
MC = N // P           # 16 m-chunks
IC = IN_DIM // P      # 2 contraction chunks for the input projection

# m-chunks whose (hp=0, q=0) multiply slice is routed ACT-cast -> GPSIMD
# (per n-half). 7 per half is the measured optimum (g=12/16/18 and the
# even-mc set all cost 1-8us). The cast is emitted right after
# tanh(mc-1): the Pool multiply still has ~5us of margin before tanh(mc)
# needs its z slice, and ACT never queues a cast whose PE slot isn't
# ready ahead of a tanh that is.
POOL_MCS = (1, 3, 5, 7, 9, 11, 13)

_CACHE = {}


def build_graph():
    nc = bacc.Bacc("TRN2", target_bir_lowering=False, debug=False)

    xT_d = nc.dram_tensor("xT", [IN_DIM, N], BF16, kind="ExternalInput")
    id_d = nc.dram_tensor("ident", [P, P], BF16, kind="ExternalInput")
    adjT_d = nc.dram_tensor("adjT", [N, N], BF16, kind="ExternalInput")
    # weights: [P, IC*NH*ATN] W-part ++ [P, ATN] C^T-part, one fast DMA
    W_d = nc.dram_tensor("Wt", [P, IC * NH * ATN + ATN], BF16, kind="ExternalInput")
    b_d = nc.dram_tensor("bias", [P, 1], F32, kind="ExternalInput")
    out_d = nc.dram_tensor("out", [P, N], BF16, kind="ExternalOutput")

    with tile.TileContext(nc) as tc:
        with (
            tc.tile_pool(name="const", bufs=1) as cp,
            tc.tile_pool(name="adj", bufs=8) as adjp,
            tc.tile_pool(name="z", bufs=7) as zp,
            tc.tile_pool(name="alpha", bufs=6) as alp,
            tc.tile_pool(name="cast", bufs=3) as scp,
            tc.tile_pool(name="ps_o", bufs=2, space="PSUM") as ps_o,
            tc.tile_pool(name="ps_s", bufs=3, space="PSUM") as ps_s,
        ):
            # The ACT activation-table load costs 1.3us and the framework
            # inserts it before ACT's first instruction: give ACT a no-dep
            # dummy op up front so the load overlaps the input DMA window
            # instead of blocking the first xtT copy. (tanh/identity/copy
            # share one table set, so it loads exactly once.)
            warm = cp.tile([1, 8], BF16)
            nc.gpsimd.memset(warm[:], 0.0)
            nc.scalar.activation(warm[:], warm[:], AF.Tanh)
            # PE p-state warm-up: the PE clock ramps mid -> full only after
            # several us of accumulated busy time, and the serial prologue
            # chain (proj -> xC -> scores) otherwise runs ~2x slow (512-col
            # matmuls measure 0.58us vs 0.26us steady-state). Real-sized
            # (K=128, 512-col) dummy matmuls during the DMA window bank
            # that busy time before proj0 arrives; tiny K=1 dummies do NOT
            # trigger the ramp.
            warmbig = cp.tile([P, 512], BF16)
            nc.gpsimd.memset(warmbig[:], 0.0)
            wp = ps_s.tile([8, 512], F32, tag="s", name="warm_ps")
            for _ in range(13):
                nc.tensor.matmul(
                    wp[:], warmbig[:, :8], warmbig[:], start=True, stop=True,
                    skip_group_check=True,
                )

            # xT chunk0 heads the serial critical chain (proj -> xC ->
            # scores -> mult -> tanh): issue it FIRST on the sync queue,
            # split by contraction half so proj's c0 matmuls can start
            # ~0.4us before c1 lands. DMA issues serialize at ~0.65us each
            # per sequencer; bias/ident ride the gpsimd SWDGE queue so adj0
            # gets the third sync slot.
            # per-chunk xT tiles: proj(nq) depends only on its own chunk's
            # DMA, so no false tile-level coupling to the later bulk loads
            xT_sb = [
                cp.tile([P, IC, 512], BF16, name=f"xT_{nq}")
                for nq in range(N // 512)
            ]
            xT_src = xT_d[:].rearrange("(c p) n -> p c n", p=P)
            nc.sync.dma_start(xT_sb[0][:], xT_src[:, :, bass.ts(0, 512)])
            Wall_sb = cp.tile([P, IC * NH * ATN + ATN], BF16)
            nc.sync.dma_start(Wall_sb[:], W_d[:])
            # bias rides sync slot 3: on the gpsimd SWDGE queue it would
            # queue behind three 384KB xT bulk transfers and gate the first
            # xtT copy by ~2us
            b_sb = cp.tile([P, 1], F32)
            nc.sync.dma_start(b_sb[:], b_d[:])
            for nq in range(1, N // 512):
                nc.gpsimd.dma_start(
                    xT_sb[nq][:], xT_src[:, :, bass.ts(nq, 512)]
                )
            ident = cp.tile([P, P], BF16)
            nc.gpsimd.dma_start(ident[:], id_d[:])
            W_sb = Wall_sb[:, : IC * NH * ATN].rearrange(
                "p (c h o) -> p c h o", c=IC, h=NH
            )
            CT_sb = Wall_sb[:, IC * NH * ATN :]

            xtT = cp.tile([P, N], BF16)
            xCT = cp.tile([P, N], BF16)
            xt4 = cp.tile([P, MC, P], BF16)
            out_sb = cp.tile([P, N], BF16)

            # --- prologue pieces ---
            def emit_xtT(nq):
                # xtT[32h+o, n] = sum_i W[h,i,o] x[n,i] + b[h,o].
                # c-outer/h-inner so the 4 col-groups run concurrently in
                # the PE array; start=True on the first c-chunk per group
                # (has_written clear is per-partition-slice). The bias
                # rides on the ACT copy out of psum.
                pt = ps_s.tile([P, 1024], F32, tag="s", name=f"pj_{nq}")
                for c in range(IC):
                    for h in range(NH):
                        nc.tensor.matmul(
                            pt[bass.ts(h, ATN), :512],
                            W_sb[:, c, h, :],
                            xT_sb[nq][:, c, :],
                            start=(c == 0),
                            stop=(c == IC - 1),
                            tile_position=(0, h * ATN),
                            skip_group_check=True,
                        )
                nc.scalar.activation(
                    xtT[:, bass.ts(nq, 512)], pt[:, :512], AF.Identity, bias=b_sb[:]
                )

            def emit_xCT(nq, on_act=True):
                # xCT[32h+p, n] = sum_o C[h,p,o] xt[n,o]; diagonal 32x32
                # tiles run concurrently in distinct row+col groups.
                pt = ps_s.tile([P, 1024], F32, tag="s", name=f"xc_{nq}")
                for h in range(NH):
                    nc.tensor.matmul(
                        pt[bass.ts(h, ATN), :512],
                        CT_sb[bass.ts(h, ATN), :],
                        xtT[bass.ts(h, ATN), bass.ts(nq, 512)],
                        start=True,
                        stop=True,
                        tile_position=(h * ATN, h * ATN),
                        skip_group_check=True,
                    )
                if on_act:
                    nc.scalar.copy(xCT[:, bass.ts(nq, 512)], pt[:, :512])
                else:
                    nc.vector.tensor_copy(xCT[:, bass.ts(nq, 512)], pt[:, :512])

            def emit_xt4(g):
                # xt4[m_local, mc, f] = xt[mc*128+m_local, f]: PE transposes
                # of xtT, 4 m-chunks per psum tile (cycled through a ps_s
                # slot). Copies on DVE (bf16 psum -> bf16 sbuf, 2x path).
                pt = ps_s.tile([P, 4, P], BF16, tag="s", name=f"tr_{g}")
                for k in range(4):
                    nc.tensor.transpose(
                        pt[:, k, :], xtT[:, bass.ts(4 * g + k, P)], ident[:]
                    )
                nc.vector.tensor_copy(xt4[:, bass.ds(4 * g, 4), :], pt[:])

            # Only projection chunk 0 gates mc0's q0 scores; chunk 1 is
            # emitted between mc0's q-blocks (PROLOG_MID), and chunks 2,3
            # plus the transpose groups are spread through nh0's early
            # m-chunks where PE has slack. heads(mc) first fires at
            # iteration mc+2, so transpose group g is due by mc 4g.
            emit_xtT(0)
            emit_xCT(0)

            def _mid0():
                emit_xtT(1)
                emit_xCT(1)

            # Deferred prologue work rides the pool-routed m-chunks (DVE
            # does one less multiply there, so its copy slots in free);
            # transpose group g is due by iteration 4g+2, proj/xC chunk
            # k by the first scores m-chunk that reads xtT chunk k.
            PROLOG_MID = {0: _mid0}
            PROLOG_AT = {0: lambda: emit_xt4(0),
                         1: lambda: emit_xtT(2),
                         2: lambda: emit_xCT(2, on_act=False),
                         3: lambda: emit_xt4(1),
                         5: lambda: emit_xtT(3),
                         7: lambda: emit_xCT(3, on_act=False),
                         9: lambda: emit_xt4(2),
                         11: lambda: emit_xt4(3)}

            # --- main loop: n-half outer, m-chunks inner, tanh lagging the
            # multiplies by one m-chunk ---
            NHALF = N // 1024
            for nh in range(NHALF):
                # Two 1-bank accumulators; the q0 output copy fires as soon
                # as q0's last matmul stops, overlapping q1's.
                po_q = [
                    ps_o.tile([P, 512], F32, tag="po", name=f"po_{nh}_{q}")
                    for q in range(2)
                ]

                def emit_heads(mc, alpha, q, nh=nh):
                    for h in range(NH):
                        nc.tensor.matmul(
                            po_q[q][bass.ts(h, ATN), :],
                            xt4[:, mc, bass.ts(h, ATN)],
                            alpha[:, h, bass.ts(q, 512)],
                            start=(mc == 0),
                            stop=(mc == MC - 1),
                            tile_position=(0, h * ATN),
                            skip_group_check=True,
                        )

                zbuf = [None, None]
                abuf = [None, None]
                pool_defer = []

                def emit_tanh(mc, nh=nh):
                    alpha = alp.tile([P, NH, 1024], BF16, tag="alpha",
                                     name=f"al_{nh}_{mc}")
                    abuf[mc % 2] = alpha
                    nc.scalar.activation(alpha[:], zbuf[mc % 2][:], AF.Tanh)

                for mc in range(MC):
                    adjt = adjp.tile([P, 1024], BF16, tag="adj")
                    nc.sync.dma_start(
                        adjt[:], adjT_d[bass.ts(mc, P), bass.ds(nh * 1024, 1024)]
                    )
                    zb = zp.tile([P, NH, 1024], BF16, tag="z",
                                 name=f"zb_{nh}_{mc}")
                    zbuf[mc % 2] = zb
                    pool_mc = mc in POOL_MCS
                    # heads(mc-2) interleaved BETWEEN scores groups: its tanh
                    # landed a full m-chunk ago so it never parks waiting
                    # instrs in front of the scores stream (the 4-deep PE
                    # wait queue would head-of-line block DVE's multiply
                    # feed), and splitting the 8-matmul heads burst in two
                    # keeps the 3-slot scores window from draining.
                    # NOTE: the pool-routed slice must be produced FIRST.
                    # Producing it LAST serializes the kernel on an
                    # ACT->GPSIMD->ACT chain (+2.2us per pool mc, measured
                    # 194us); even SECOND measures ~+20us - any
                    # displacement perturbs the 3-slot ps_s rotation more
                    # than the ~4us cast slot-hold costs.
                    qhps = [(q, hp) for q in range(2) for hp in range(NH // 2)]
                    q_seen = set()
                    for q, hp in qhps:
                        if q not in q_seen:
                            q_seen.add(q)
                            if nh == 0 and q == 1 and mc in PROLOG_MID:
                                PROLOG_MID[mc]()
                            if mc >= 2:
                                emit_heads(mc - 2, abuf[mc % 2], q)
                        s2 = ps_s.tile([P, 2, 512], F32, tag="s")
                        for j in range(2):
                            h = 2 * hp + j
                            nc.tensor.matmul(
                                s2[:, j, :],
                                xtT[bass.ts(h, ATN), bass.ts(mc, P)],
                                xCT[
                                    bass.ts(h, ATN),
                                    bass.ds(nh * 1024 + q * 512, 512),
                                ],
                                start=True,
                                stop=True,
                                tile_position=(h * ATN, 0),
                                skip_group_check=True,
                            )
                        zsl = zb[:, bass.ds(2 * hp, 2), bass.ts(q, 512)]
                        adj_b = adjt[:, None, bass.ts(q, 512)].to_broadcast(
                            (P, 2, 512)
                        )
                        if pool_mc and q == 0 and hp == 0:
                            # deferred: ACT casts psum->bf16 after
                            # tanh(mc-1); the otherwise-idle GPSIMD
                            # engine does the multiply (it has no PSUM
                            # port, hence the cast).
                            pool_defer.append((s2, zsl, adj_b))
                        else:
                            nc.vector.tensor_tensor(zsl, s2[:], adj_b, ALU.mult)
                    if nh == 0 and mc in PROLOG_AT:
                        PROLOG_AT[mc]()
                    if mc >= 1:
                        emit_tanh(mc - 1)
                    for (s2, zsl, adj_b) in pool_defer:
                        sc = scp.tile([P, 2, 512], BF16, tag="cast")
                        nc.scalar.copy(sc[:], s2[:])
                        nc.gpsimd.tensor_tensor(zsl, sc[:], adj_b, ALU.mult)
                    pool_defer.clear()
                # tail: heads(14), then per-q tanh(15) + heads(15) + output
                # copy + DMA so the last 512-col DMA chases the last matmul
                for q in range(2):
                    emit_heads(MC - 2, abuf[(MC - 2) % 2], q)
                alpha = alp.tile([P, NH, 1024], BF16, tag="alpha",
                                 name=f"al_{nh}_{MC - 1}")
                for q in range(2):
                    nc.scalar.activation(
                        alpha[:, :, bass.ts(q, 512)],
                        zbuf[(MC - 1) % 2][:, :, bass.ts(q, 512)],
                        AF.Tanh,
                    )
                    emit_heads(MC - 1, alpha, q)
                    if nh == NHALF - 1 and q == 1:
                        # very last output block: split the copy across
                        # ACT+DVE halves with two chasing DMAs to shorten
                        # the serial tail
                        for half, eng in ((0, nc.scalar.copy),
                                          (1, nc.vector.tensor_copy)):
                            off = nh * 1024 + q * 512 + half * 256
                            eng(out_sb[:, bass.ds(off, 256)],
                                po_q[q][:, bass.ts(half, 256)])
                            nc.sync.dma_start(
                                out_d[:, bass.ds(off, 256)],
                                out_sb[:, bass.ds(off, 256)],
                            )
                    else:
                        nc.vector.tensor_copy(
                            out_sb[:, bass.ds(nh * 1024 + q * 512, 512)],
                            po_q[q][:],
                        )
                        nc.sync.dma_start(
                            out_d[:, bass.ds(nh * 1024 + q * 512, 512)],
                            out_sb[:, bass.ds(nh * 1024 + q * 512, 512)],
                        )

    nc.compile()
    return nc


def _get_graph():
    if "nc" not in _CACHE:
        _CACHE["nc"] = build_graph()
    return _CACHE["nc"]


def make_in_maps(x, adj, W, b, C):
    in_maps = []
    for core in range(NCORES):
        bb = core // 2
        hg = core % 2
        hs = slice(hg * NH, (hg + 1) * NH)
        Wt = (
            W[hs]
            .reshape(NH, IC, P, ATN)
            .transpose(2, 1, 0, 3)
            .reshape(P, IC * NH * ATN)
        )
        CTt = C[hs].transpose(0, 2, 1).reshape(NH * ATN, ATN)
        in_maps.append(
            {
                "xT": np.ascontiguousarray(x[bb].T).astype(BF16_NP),
                "ident": np.eye(P, dtype=np.float32).astype(BF16_NP),
                "adjT": np.ascontiguousarray(adj[bb].T).astype(BF16_NP),
                "Wt": np.ascontiguousarray(
                    np.concatenate([Wt, CTt], axis=1)
                ).astype(BF16_NP),
                "bias": np.ascontiguousarray(b[hs].reshape(P, 1)),
            }
        )
    return in_maps


LAST_RESULT = None


def kernel(x, adj, W, b, C):
    global LAST_RESULT
    x = np.asarray(x, dtype=np.float32)
    adj = np.asarray(adj, dtype=np.float32)
    W = np.asarray(W, dtype=np.float32)
    b = np.asarray(b, dtype=np.float32)
    C = np.asarray(C, dtype=np.float32)

    nc = _get_graph()
    in_maps = make_in_maps(x, adj, W, b, C)
    res = run_bass_kernel_spmd(nc, in_maps, core_ids=list(range(NCORES)))
    LAST_RESULT = res

    out = np.empty((B, N, H * ATN), dtype=np.float32)
    for core in range(NCORES):
        bb = core // 2
        hg = core % 2
        out[bb, :, hg * P : (hg + 1) * P] = (
            res.results[core]["out"].astype(np.float32).T
        )
    return out


# revision 64
# speedup vs baseline: 1.0093x; 1.0068x over previous
"""Trainium2 Bass kernel for per-head bilinear graph attention.

Reference computation (B=4, N=2048, IN=256, H=8, ATN=32):
    xt     = einsum('bni,hio->bhno', x, W) + b          # [B,H,N,32]
    xC     = einsum('bhno,hpo->bhnp', xt, C)            # [B,H,N,32]
    scores = einsum('bhnp,bhmp->bhnm', xC, xt)          # [B,H,N,N]
    alpha  = tanh(scores * adj[:,None])                 # [B,H,N,N]
    heads  = einsum('bhnm,bhmo->bhno', alpha, xt)       # [B,H,N,32]
    out    = concat heads on feature dim                # [B,N,256]

Sharding: 8 cores = 4 batches x 2 head-groups (4 heads each). Fully
data-parallel, no collectives. Each core computes out[b, :, hg*128:(hg+1)*128]
transposed ([128, 2048]); the host transposes back and concatenates.

Device-side layout is fully transposed ("T" = [feature/m, n]):
    xtT  [128(4h x 32o), 2048n]   stacked per-head xt^T (bias included)
    xCT  [128(4h x 32p), 2048n]   stacked per-head xC^T
    sT   [128m, n]     = scores[n, m]   (psum, per m-chunk per head)
    z    = sT * adjT   (adjT host-pretransposed so it is [m, n])
    alphaT = tanh(z)
    outT [128(4h x 32o), 2048n] accumulated in psum over 16 m-chunks

Engine budget per core: the kernel sits at the DVE+ACT joint capacity
floor (TRN2 matmul psum output is fp32-only, so every score element must
exit PSUM through a 1x fp32 DVE/ACT op; tanh is ACT-only at 0.83ns/col).
The LP optimum routes 14 of the 128 multiply units through an ACT cast +
GPSIMD multiply and fuses the rest with the psum exit on DVE: ACT ~=
137.8us, DVE ~= 135.5us busy; measured exec 167.6-170.6us at nominal
clock, median ~168.8 (vs 179.5 for the paired-tanh v1 at the same
clock; runs draw a +-20% clock lottery, compare via exec / TANH-busy).
Remaining overhead is ~6.7us fixed NEFF launch, ~8us serial prologue
chain (DMA -> proj -> xC -> scores, latency-tight at each hop), ~5us
tail, ~12us three-engine phase jitter. The scheduling rules that
matter, each worth us-level regressions when violated:
 - per-mc z/alpha tiles [P,4,1024] with tanh pipelined ONE m-chunk
   behind the multiplies (tanh(mc-1) after mc's scores+mults emission),
   deep rotations (z 7, alpha 6, adj 8) so phase jitter never couples
   the engines.
 - heads(mc-2) matmuls interleave BETWEEN the two scores q-groups: a
   contiguous 8-matmul heads burst drains the 3-slot scores window and
   starves DVE ~1.2us per 2 mc; parking heads in front of scores
   head-of-line blocks PE's 4-deep wait queue on ACT's tanh.
 - pool casts are emitted AFTER tanh(mc-1) so ACT never queues a cast
   whose PE slot isn't written yet ahead of a ready tanh; the GPSIMD
   multiply still has ~5us of margin before tanh(mc) needs its slice.
 - no zero-seed matmuls: has_written clears are per-partition-slice on
   this HW (the v1 diagonal xCT matmuls with start=True already relied
   on that), so heads/proj groups use start=(first k-chunk).
 - prologue: a no-dep dummy ACT op hoists the 1.3us activation-table
   load into the DMA window (it otherwise lands between proj0 and the
   first xtT copy); 13 real-sized (K=128, 512-col) dummy matmuls bank
   PE busy time so the mid->full p-state ramp happens inside the DMA
   window instead of stretching the proj/xC/scores chain 2x (tiny K=1
   dummies do NOT trigger the ramp). xT is loaded into per-chunk tiles
   so proj0 depends only on its own DMA; sync queue order xT0, W, bias,
   then adj (bias on the gpsimd SWDGE queue queues behind the 384KB xT
   bulk transfers and gates the first xtT copy ~2us). Only proj/xC
   chunk 0 gates mc0's q0 scores; chunk 1 lands between mc0's q-blocks,
   chunks 2,3 + the xt4 transpose groups ride pool-routed m-chunks
   where DVE has slack.
 - tail: mc15's tanh+heads+output copy are emitted per-q, and the very
   last 512-col block splits its copy across ACT+DVE halves with two
   chasing DMAs.

Key facts baked into this design (from HW traces + the CoreSim cost
model source):
 - DVE tensor_tensor 2x mode needs ALL operands 2-byte; fp32 psum scores
   therefore pin the mask-multiply at 1x. TRN2 matmul cannot write bf16
   psum (is_transpose=True really is a different PE datapath - measured
   garbage), and DMA has no PSUM port, so there is no cheaper exit.
 - GPSIMD cannot access PSUM; its multiplies need an ACT cast first
   (1.0us/unit ACT buys 1.15us/unit off DVE - profitable only while
   ACT's tanh load leaves it slack).
 - K=32 scores matmuls are packed pairwise into PE row-groups
   (tile_position) with [128,2,512] psum tiles; outT uses 4-way
   col-group packing. PSUM: 3x2-bank scores slots + 2x1-bank output
   accumulators = 8 banks.
"""

import sys
import types

import numpy as np
import ml_dtypes

BF16_NP = ml_dtypes.bfloat16


def _ensure_axon_ntff_hook():
    """Provide antenv.axon_hooks if the image lacks it, so
    run_bass_kernel_spmd(trace=True) can capture NTFF profiles instead of
    crashing on the import. No-op when the real module exists."""
    try:
        import antenv.axon_hooks  # noqa: F401

        return
    except ImportError:
        pass
    mod = types.ModuleType("antenv.axon_hooks")
    _state = {"hook": None}
    mod.set_axon_ntff_profile_hook = lambda h: _state.__setitem__("hook", h)
    mod.get_axon_ntff_profile_hook = lambda: _state["hook"]
    sys.modules["antenv.axon_hooks"] = mod
    try:
        import antenv

        antenv.axon_hooks = mod
    except ImportError:
        pass
    try:
        from trn_agent_boot.trn_boot import _ntff_profile_via_ctypes

        mod.set_axon_ntff_profile_hook(
            _ntff_profile_via_ctypes("/opt/axon/libaxon_pjrt.so")
        )
    except Exception:
        pass


_ensure_axon_ntff_hook()

from concourse import bacc, mybir, tile
import concourse.bass as bass
from concourse.bass_utils import run_bass_kernel_spmd

F32 = mybir.dt.float32
BF16 = mybir.dt.bfloat16
AF = mybir.ActivationFunctionType
ALU = mybir.AluOpType

P = 128
B, N, IN_DIM, H, ATN = 4, 2048, 256, 8, 32
NH = 4                # heads per core
NCORES = 8
MC = N // P           # 16 m-chunks
IC = IN_DIM // P      # 2 contraction chunks for the input projection

# m-chunks whose (hp=0, q=0) multiply slice is routed ACT-cast -> GPSIMD
# (per n-half). 7 per half is the measured optimum (g=12/16/18 and the
# even-mc set all cost 1-8us). The cast is emitted right after
# tanh(mc-1): the Pool multiply still has ~5us of margin before tanh(mc)
# needs its z slice, and ACT never queues a cast whose PE slot isn't
# ready ahead of a tanh that is.
POOL_MCS = (1, 3, 5, 7, 9, 11, 13)
# nh1 runs lighter on DVE (no deferred prologue copies) and shows
# periodic DVE idle: it carries 2 fewer pool casts, saving the pacing
# ACT engine 2us while nh1's DVE absorbs the extra multiplies.
POOL_MCS_NH = (POOL_MCS, (3, 5, 7, 9, 11))

_CACHE = {}


def build_graph():
    nc = bacc.Bacc("TRN2", target_bir_lowering=False, debug=False)

    xT_d = nc.dram_tensor("xT", [IN_DIM, N], BF16, kind="ExternalInput")
    id_d = nc.dram_tensor("ident", [P, P], BF16, kind="ExternalInput")
    adjT_d = nc.dram_tensor("adjT", [N, N], BF16, kind="ExternalInput")
    # weights: [P, IC*NH*ATN] W-part ++ [P, ATN] C^T-part, one fast DMA
    W_d = nc.dram_tensor("Wt", [P, IC * NH * ATN + ATN], BF16, kind="ExternalInput")
    b_d = nc.dram_tensor("bias", [P, 1], F32, kind="ExternalInput")
    out_d = nc.dram_tensor("out", [P, N], BF16, kind="ExternalOutput")

    with tile.TileContext(nc) as tc:
        with (
            tc.tile_pool(name="const", bufs=1) as cp,
            tc.tile_pool(name="adj", bufs=8) as adjp,
            tc.tile_pool(name="z", bufs=7) as zp,
            tc.tile_pool(name="alpha", bufs=6) as alp,
            tc.tile_pool(name="cast", bufs=3) as scp,
            tc.tile_pool(name="ps_o", bufs=2, space="PSUM") as ps_o,
            tc.tile_pool(name="ps_s", bufs=3, space="PSUM") as ps_s,
        ):
            # The ACT activation-table load costs 1.3us and the framework
            # inserts it before ACT's first instruction: give ACT a no-dep
            # dummy op up front so the load overlaps the input DMA window
            # instead of blocking the first xtT copy. (tanh/identity/copy
            # share one table set, so it loads exactly once.)
            warm = cp.tile([1, 8], BF16)
            nc.gpsimd.memset(warm[:], 0.0)
            nc.scalar.activation(warm[:], warm[:], AF.Tanh)
            # PE p-state warm-up: the PE clock ramps mid -> full only after
            # several us of accumulated busy time, and the serial prologue
            # chain (proj -> xC -> scores) otherwise runs ~2x slow (512-col
            # matmuls measure 0.58us vs 0.26us steady-state). Real-sized
            # (K=128, 512-col) dummy matmuls during the DMA window bank
            # that busy time before proj0 arrives; tiny K=1 dummies do NOT
            # trigger the ramp.
            warmbig = cp.tile([P, 512], BF16)
            nc.gpsimd.memset(warmbig[:], 0.0)
            wp = ps_s.tile([8, 512], F32, tag="s", name="warm_ps")
            for _ in range(13):
                nc.tensor.matmul(
                    wp[:], warmbig[:, :8], warmbig[:], start=True, stop=True,
                    skip_group_check=True,
                )

            # xT chunk0 heads the serial critical chain (proj -> xC ->
            # scores -> mult -> tanh): issue it FIRST on the sync queue,
            # split by contraction half so proj's c0 matmuls can start
            # ~0.4us before c1 lands. DMA issues serialize at ~0.65us each
            # per sequencer; bias/ident ride the gpsimd SWDGE queue so adj0
            # gets the third sync slot.
            # per-chunk xT tiles: proj(nq) depends only on its own chunk's
            # DMA, so no false tile-level coupling to the later bulk loads
            xT_sb = [
                cp.tile([P, IC, 512], BF16, name=f"xT_{nq}")
                for nq in range(N // 512)
            ]
            xT_src = xT_d[:].rearrange("(c p) n -> p c n", p=P)
            nc.sync.dma_start(xT_sb[0][:], xT_src[:, :, bass.ts(0, 512)])
            Wall_sb = cp.tile([P, IC * NH * ATN + ATN], BF16)
            nc.sync.dma_start(Wall_sb[:], W_d[:])
            # bias rides sync slot 3: on the gpsimd SWDGE queue it would
            # queue behind three 384KB xT bulk transfers and gate the first
            # xtT copy by ~2us
            b_sb = cp.tile([P, 1], F32)
            nc.sync.dma_start(b_sb[:], b_d[:])
            for nq in range(1, N // 512):
                nc.gpsimd.dma_start(
                    xT_sb[nq][:], xT_src[:, :, bass.ts(nq, 512)]
                )
            ident = cp.tile([P, P], BF16)
            nc.gpsimd.dma_start(ident[:], id_d[:])
            W_sb = Wall_sb[:, : IC * NH * ATN].rearrange(
                "p (c h o) -> p c h o", c=IC, h=NH
            )
            CT_sb = Wall_sb[:, IC * NH * ATN :]

            xtT = cp.tile([P, N], BF16)
            xCT = cp.tile([P, N], BF16)
            xt4 = cp.tile([P, MC, P], BF16)
            out_sb = cp.tile([P, N], BF16)

            # --- prologue pieces ---
            def emit_xtT(nq):
                # xtT[32h+o, n] = sum_i W[h,i,o] x[n,i] + b[h,o].
                # c-outer/h-inner so the 4 col-groups run concurrently in
                # the PE array; start=True on the first c-chunk per group
                # (has_written clear is per-partition-slice). The bias
                # rides on the ACT copy out of psum.
                pt = ps_s.tile([P, 1024], F32, tag="s", name=f"pj_{nq}")
                for c in range(IC):
                    for h in range(NH):
                        nc.tensor.matmul(
                            pt[bass.ts(h, ATN), :512],
                            W_sb[:, c, h, :],
                            xT_sb[nq][:, c, :],
                            start=(c == 0),
                            stop=(c == IC - 1),
                            tile_position=(0, h * ATN),
                            skip_group_check=True,
                        )
                nc.scalar.activation(
                    xtT[:, bass.ts(nq, 512)], pt[:, :512], AF.Identity, bias=b_sb[:]
                )

            def emit_xCT(nq, on_act=True):
                # xCT[32h+p, n] = sum_o C[h,p,o] xt[n,o]; diagonal 32x32
                # tiles run concurrently in distinct row+col groups.
                pt = ps_s.tile([P, 1024], F32, tag="s", name=f"xc_{nq}")
                for h in range(NH):
                    nc.tensor.matmul(
                        pt[bass.ts(h, ATN), :512],
                        CT_sb[bass.ts(h, ATN), :],
                        xtT[bass.ts(h, ATN), bass.ts(nq, 512)],
                        start=True,
                        stop=True,
                        tile_position=(h * ATN, h * ATN),
                        skip_group_check=True,
                    )
                if on_act:
                    nc.scalar.copy(xCT[:, bass.ts(nq, 512)], pt[:, :512])
                else:
                    nc.vector.tensor_copy(xCT[:, bass.ts(nq, 512)], pt[:, :512])

            def emit_xt4(g):
                # xt4[m_local, mc, f] = xt[mc*128+m_local, f]: PE transposes
                # of xtT, 4 m-chunks per psum tile (cycled through a ps_s
                # slot). Copies on DVE (bf16 psum -> bf16 sbuf, 2x path).
                pt = ps_s.tile([P, 4, P], BF16, tag="s", name=f"tr_{g}")
                for k in range(4):
                    nc.tensor.transpose(
                        pt[:, k, :], xtT[:, bass.ts(4 * g + k, P)], ident[:]
                    )
                nc.vector.tensor_copy(xt4[:, bass.ds(4 * g, 4), :], pt[:])

            # Only projection chunk 0 gates mc0's q0 scores; chunk 1 is
            # emitted between mc0's q-blocks (PROLOG_MID), and chunks 2,3
            # plus the transpose groups are spread through nh0's early
            # m-chunks where PE has slack. heads(mc) first fires at
            # iteration mc+2, so transpose group g is due by mc 4g.
            emit_xtT(0)
            emit_xCT(0)

            def _mid0():
                emit_xtT(1)
                emit_xCT(1)

            # Deferred prologue work rides the pool-routed m-chunks (DVE
            # does one less multiply there, so its copy slots in free);
            # transpose group g is due by iteration 4g+2, proj/xC chunk
            # k by the first scores m-chunk that reads xtT chunk k.
            PROLOG_MID = {0: _mid0}
            PROLOG_AT = {0: lambda: emit_xt4(0),
                         1: lambda: emit_xtT(2),
                         2: lambda: emit_xCT(2, on_act=False),
                         3: lambda: emit_xt4(1),
                         5: lambda: emit_xtT(3),
                         7: lambda: emit_xCT(3, on_act=False),
                         9: lambda: emit_xt4(2),
                         11: lambda: emit_xt4(3)}

            # --- main loop: n-half outer, m-chunks inner, tanh lagging the
            # multiplies by one m-chunk ---
            NHALF = N // 1024
            for nh in range(NHALF):
                # Two 1-bank accumulators; the q0 output copy fires as soon
                # as q0's last matmul stops, overlapping q1's.
                po_q = [
                    ps_o.tile([P, 512], F32, tag="po", name=f"po_{nh}_{q}")
                    for q in range(2)
                ]

                def emit_heads(mc, alpha, q, nh=nh):
                    for h in range(NH):
                        nc.tensor.matmul(
                            po_q[q][bass.ts(h, ATN), :],
                            xt4[:, mc, bass.ts(h, ATN)],
                            alpha[:, h, bass.ts(q, 512)],
                            start=(mc == 0),
                            stop=(mc == MC - 1),
                            tile_position=(0, h * ATN),
                            skip_group_check=True,
                        )

                zbuf = [None, None]
                abuf = [None, None]
                pool_defer = []

                def emit_tanh(mc, nh=nh):
                    alpha = alp.tile([P, NH, 1024], BF16, tag="alpha",
                                     name=f"al_{nh}_{mc}")
                    abuf[mc % 2] = alpha
                    nc.scalar.activation(alpha[:], zbuf[mc % 2][:], AF.Tanh)

                for mc in range(MC):
                    adjt = adjp.tile([P, 1024], BF16, tag="adj")
                    nc.sync.dma_start(
                        adjt[:], adjT_d[bass.ts(mc, P), bass.ds(nh * 1024, 1024)]
                    )
                    zb = zp.tile([P, NH, 1024], BF16, tag="z",
                                 name=f"zb_{nh}_{mc}")
                    zbuf[mc % 2] = zb
                    pool_mc = mc in POOL_MCS_NH[nh]
                    # heads(mc-2) interleaved BETWEEN scores groups: its tanh
                    # landed a full m-chunk ago so it never parks waiting
                    # instrs in front of the scores stream (the 4-deep PE
                    # wait queue would head-of-line block DVE's multiply
                    # feed), and splitting the 8-matmul heads burst in two
                    # keeps the 3-slot scores window from draining.
                    # NOTE: the pool-routed slice must be produced FIRST.
                    # Producing it LAST serializes the kernel on an
                    # ACT->GPSIMD->ACT chain (+2.2us per pool mc, measured
                    # 194us); even SECOND measures ~+20us - any
                    # displacement perturbs the 3-slot ps_s rotation more
                    # than the ~4us cast slot-hold costs.
                    qhps = [(q, hp) for q in range(2) for hp in range(NH // 2)]
                    q_seen = set()
                    for q, hp in qhps:
                        if q not in q_seen:
                            q_seen.add(q)
                            if nh == 0 and q == 1 and mc in PROLOG_MID:
                                PROLOG_MID[mc]()
                            if mc >= 2:
                                emit_heads(mc - 2, abuf[mc % 2], q)
                        s2 = ps_s.tile([P, 2, 512], F32, tag="s")
                        for j in range(2):
                            h = 2 * hp + j
                            nc.tensor.matmul(
                                s2[:, j, :],
                                xtT[bass.ts(h, ATN), bass.ts(mc, P)],
                                xCT[
                                    bass.ts(h, ATN),
                                    bass.ds(nh * 1024 + q * 512, 512),
                                ],
                                start=True,
                                stop=True,
                                tile_position=(h * ATN, 0),
                                skip_group_check=True,
                            )
                        zsl = zb[:, bass.ds(2 * hp, 2), bass.ts(q, 512)]
                        adj_b = adjt[:, None, bass.ts(q, 512)].to_broadcast(
                            (P, 2, 512)
                        )
                        if pool_mc and q == 0 and hp == 0:
                            # deferred: ACT casts psum->bf16 after
                            # tanh(mc-1); the otherwise-idle GPSIMD
                            # engine does the multiply (it has no PSUM
                            # port, hence the cast).
                            pool_defer.append((s2, zsl, adj_b))
                        else:
                            nc.vector.tensor_tensor(zsl, s2[:], adj_b, ALU.mult)
                    if nh == 0 and mc in PROLOG_AT:
                        PROLOG_AT[mc]()
                    if mc >= 1:
                        emit_tanh(mc - 1)
                    for (s2, zsl, adj_b) in pool_defer:
                        sc = scp.tile([P, 2, 512], BF16, tag="cast")
                        nc.scalar.copy(sc[:], s2[:])
                        nc.gpsimd.tensor_tensor(zsl, sc[:], adj_b, ALU.mult)
                    pool_defer.clear()
                # tail: heads(14), then per-q tanh(15) + heads(15) + output
                # copy + DMA so the last 512-col DMA chases the last matmul
                for q in range(2):
                    emit_heads(MC - 2, abuf[(MC - 2) % 2], q)
                alpha = alp.tile([P, NH, 1024], BF16, tag="alpha",
                                 name=f"al_{nh}_{MC - 1}")
                for q in range(2):
                    nc.scalar.activation(
                        alpha[:, :, bass.ts(q, 512)],
                        zbuf[(MC - 1) % 2][:, :, bass.ts(q, 512)],
                        AF.Tanh,
                    )
                    emit_heads(MC - 1, alpha, q)
                    if nh == NHALF - 1 and q == 1:
                        # very last output block: split the copy across
                        # ACT+DVE halves with two chasing DMAs to shorten
                        # the serial tail
                        for half, eng in ((0, nc.scalar.copy),
                                          (1, nc.vector.tensor_copy)):
                            off = nh * 1024 + q * 512 + half * 256
                            eng(out_sb[:, bass.ds(off, 256)],
                                po_q[q][:, bass.ts(half, 256)])
                            nc.sync.dma_start(
                                out_d[:, bass.ds(off, 256)],
                                out_sb[:, bass.ds(off, 256)],
                            )
                    else:
                        nc.vector.tensor_copy(
                            out_sb[:, bass.ds(nh * 1024 + q * 512, 512)],
                            po_q[q][:],
                        )
                        nc.sync.dma_start(
                            out_d[:, bass.ds(nh * 1024 + q * 512, 512)],
                            out_sb[:, bass.ds(nh * 1024 + q * 512, 512)],
                        )

    nc.compile()
    return nc


def _get_graph():
    if "nc" not in _CACHE:
        _CACHE["nc"] = build_graph()
    return _CACHE["nc"]


def make_in_maps(x, adj, W, b, C):
    in_maps = []
    for core in range(NCORES):
        bb = core // 2
        hg = core % 2
        hs = slice(hg * NH, (hg + 1) * NH)
        Wt = (
            W[hs]
            .reshape(NH, IC, P, ATN)
            .transpose(2, 1, 0, 3)
            .reshape(P, IC * NH * ATN)
        )
        CTt = C[hs].transpose(0, 2, 1).reshape(NH * ATN, ATN)
        in_maps.append(
            {
                "xT": np.ascontiguousarray(x[bb].T).astype(BF16_NP),
                "ident": np.eye(P, dtype=np.float32).astype(BF16_NP),
                "adjT": np.ascontiguousarray(adj[bb].T).astype(BF16_NP),
                "Wt": np.ascontiguousarray(
                    np.concatenate([Wt, CTt], axis=1)
                ).astype(BF16_NP),
                "bias": np.ascontiguousarray(b[hs].reshape(P, 1)),
            }
        )
    return in_maps


LAST_RESULT = None


def kernel(x, adj, W, b, C):
    global LAST_RESULT
    x = np.asarray(x, dtype=np.float32)
    adj = np.asarray(adj, dtype=np.float32)
    W = np.asarray(W, dtype=np.float32)
    b = np.asarray(b, dtype=np.float32)
    C = np.asarray(C, dtype=np.float32)

    nc = _get_graph()
    in_maps = make_in_maps(x, adj, W, b, C)
    res = run_bass_kernel_spmd(nc, in_maps, core_ids=list(range(NCORES)))
    LAST_RESULT = res

    out = np.empty((B, N, H * ATN), dtype=np.float32)
    for core in range(NCORES):
        bb = core // 2
        hg = core % 2
        out[bb, :, hg * P : (hg + 1) * P] = (
            res.results[core]["out"].astype(np.float32).T
        )
    return out


# revision 65
# speedup vs baseline: 1.0096x; 1.0003x over previous
"""Trainium2 Bass kernel for per-head bilinear graph attention.

Reference computation (B=4, N=2048, IN=256, H=8, ATN=32):
    xt     = einsum('bni,hio->bhno', x, W) + b          # [B,H,N,32]
    xC     = einsum('bhno,hpo->bhnp', xt, C)            # [B,H,N,32]
    scores = einsum('bhnp,bhmp->bhnm', xC, xt)          # [B,H,N,N]
    alpha  = tanh(scores * adj[:,None])                 # [B,H,N,N]
    heads  = einsum('bhnm,bhmo->bhno', alpha, xt)       # [B,H,N,32]
    out    = concat heads on feature dim                # [B,N,256]

Sharding: 8 cores = 4 batches x 2 head-groups (4 heads each). Fully
data-parallel, no collectives. Each core computes out[b, :, hg*128:(hg+1)*128]
transposed ([128, 2048]); the host transposes back and concatenates.

Device-side layout is fully transposed ("T" = [feature/m, n]):
    xtT  [128(4h x 32o), 2048n]   stacked per-head xt^T (bias included)
    xCT  [128(4h x 32p), 2048n]   stacked per-head xC^T
    sT   [128m, n]     = scores[n, m]   (psum, per m-chunk per head)
    z    = sT * adjT   (adjT host-pretransposed so it is [m, n])
    alphaT = tanh(z)
    outT [128(4h x 32o), 2048n] accumulated in psum over 16 m-chunks

Engine budget per core: the kernel sits at the DVE+ACT joint capacity
floor (TRN2 matmul psum output is fp32-only, so every score element must
exit PSUM through a 1x fp32 DVE/ACT op; tanh is ACT-only at 0.83ns/col).
The LP optimum routes 14 of the 128 multiply units through an ACT cast +
GPSIMD multiply and fuses the rest with the psum exit on DVE: ACT ~=
137.8us, DVE ~= 135.5us busy; measured exec 167.6-170.6us at nominal
clock, median ~168.8 (vs 179.5 for the paired-tanh v1 at the same
clock; runs draw a +-20% clock lottery, compare via exec / TANH-busy).
Remaining overhead is ~6.7us fixed NEFF launch, ~8us serial prologue
chain (DMA -> proj -> xC -> scores, latency-tight at each hop), ~5us
tail, ~12us three-engine phase jitter. The scheduling rules that
matter, each worth us-level regressions when violated:
 - per-mc z/alpha tiles [P,4,1024] with tanh pipelined ONE m-chunk
   behind the multiplies (tanh(mc-1) after mc's scores+mults emission),
   deep rotations (z 7, alpha 6, adj 8) so phase jitter never couples
   the engines.
 - heads(mc-2) matmuls interleave BETWEEN the two scores q-groups: a
   contiguous 8-matmul heads burst drains the 3-slot scores window and
   starves DVE ~1.2us per 2 mc; parking heads in front of scores
   head-of-line blocks PE's 4-deep wait queue on ACT's tanh.
 - pool casts are emitted AFTER tanh(mc-1) so ACT never queues a cast
   whose PE slot isn't written yet ahead of a ready tanh; the GPSIMD
   multiply still has ~5us of margin before tanh(mc) needs its slice.
 - no zero-seed matmuls: has_written clears are per-partition-slice on
   this HW (the v1 diagonal xCT matmuls with start=True already relied
   on that), so heads/proj groups use start=(first k-chunk).
 - prologue: a no-dep dummy ACT op hoists the 1.3us activation-table
   load into the DMA window (it otherwise lands between proj0 and the
   first xtT copy); 13 real-sized (K=128, 512-col) dummy matmuls bank
   PE busy time so the mid->full p-state ramp happens inside the DMA
   window instead of stretching the proj/xC/scores chain 2x (tiny K=1
   dummies do NOT trigger the ramp). xT is loaded into per-chunk tiles
   so proj0 depends only on its own DMA; sync queue order xT0, W, bias,
   then adj (bias on the gpsimd SWDGE queue queues behind the 384KB xT
   bulk transfers and gates the first xtT copy ~2us). Only proj/xC
   chunk 0 gates mc0's q0 scores; chunk 1 lands between mc0's q-blocks,
   chunks 2,3 + the xt4 transpose groups ride pool-routed m-chunks
   where DVE has slack.
 - tail: mc15's tanh+heads+output copy are emitted per-q, and the very
   last 512-col block splits its copy across ACT+DVE halves with two
   chasing DMAs.

Key facts baked into this design (from HW traces + the CoreSim cost
model source):
 - DVE tensor_tensor 2x mode needs ALL operands 2-byte; fp32 psum scores
   therefore pin the mask-multiply at 1x. TRN2 matmul cannot write bf16
   psum (is_transpose=True really is a different PE datapath - measured
   garbage), and DMA has no PSUM port, so there is no cheaper exit.
 - GPSIMD cannot access PSUM; its multiplies need an ACT cast first
   (1.0us/unit ACT buys 1.15us/unit off DVE - profitable only while
   ACT's tanh load leaves it slack).
 - K=32 scores matmuls are packed pairwise into PE row-groups
   (tile_position) with [128,2,512] psum tiles; outT uses 4-way
   col-group packing. PSUM: 3x2-bank scores slots + 2x1-bank output
   accumulators = 8 banks.
"""

import sys
import types

import numpy as np
import ml_dtypes

BF16_NP = ml_dtypes.bfloat16


def _ensure_axon_ntff_hook():
    """Provide antenv.axon_hooks if the image lacks it, so
    run_bass_kernel_spmd(trace=True) can capture NTFF profiles instead of
    crashing on the import. No-op when the real module exists."""
    try:
        import antenv.axon_hooks  # noqa: F401

        return
    except ImportError:
        pass
    mod = types.ModuleType("antenv.axon_hooks")
    _state = {"hook": None}
    mod.set_axon_ntff_profile_hook = lambda h: _state.__setitem__("hook", h)
    mod.get_axon_ntff_profile_hook = lambda: _state["hook"]
    sys.modules["antenv.axon_hooks"] = mod
    try:
        import antenv

        antenv.axon_hooks = mod
    except ImportError:
        pass
    try:
        from trn_agent_boot.trn_boot import _ntff_profile_via_ctypes

        mod.set_axon_ntff_profile_hook(
            _ntff_profile_via_ctypes("/opt/axon/libaxon_pjrt.so")
        )
    except Exception:
        pass


_ensure_axon_ntff_hook()

from concourse import bacc, mybir, tile
import concourse.bass as bass
from concourse.bass_utils import run_bass_kernel_spmd

F32 = mybir.dt.float32
BF16 = mybir.dt.bfloat16
AF = mybir.ActivationFunctionType
ALU = mybir.AluOpType

P = 128
B, N, IN_DIM, H, ATN = 4, 2048, 256, 8, 32
NH = 4                # heads per core
NCORES = 8
MC = N // P           # 16 m-chunks
IC = IN_DIM // P      # 2 contraction chunks for the input projection

# m-chunks whose (hp=0, q=0) multiply slice is routed ACT-cast -> GPSIMD
# (per n-half). 7 per half is the measured optimum (g=12/16/18 and the
# even-mc set all cost 1-8us). The cast is emitted right after
# tanh(mc-1): the Pool multiply still has ~5us of margin before tanh(mc)
# needs its z slice, and ACT never queues a cast whose PE slot isn't
# ready ahead of a tanh that is.
POOL_MCS = (1, 3, 5, 7, 9, 11, 13)

_CACHE = {}


def build_graph():
    nc = bacc.Bacc("TRN2", target_bir_lowering=False, debug=False)

    xT_d = nc.dram_tensor("xT", [IN_DIM, N], BF16, kind="ExternalInput")
    id_d = nc.dram_tensor("ident", [P, P], BF16, kind="ExternalInput")
    adjT_d = nc.dram_tensor("adjT", [N, N], BF16, kind="ExternalInput")
    # weights: [P, IC*NH*ATN] W-part ++ [P, ATN] C^T-part, one fast DMA
    W_d = nc.dram_tensor("Wt", [P, IC * NH * ATN + ATN], BF16, kind="ExternalInput")
    b_d = nc.dram_tensor("bias", [P, 1], F32, kind="ExternalInput")
    out_d = nc.dram_tensor("out", [P, N], BF16, kind="ExternalOutput")

    with tile.TileContext(nc) as tc:
        with (
            tc.tile_pool(name="const", bufs=1) as cp,
            tc.tile_pool(name="adj", bufs=8) as adjp,
            tc.tile_pool(name="z", bufs=7) as zp,
            tc.tile_pool(name="alpha", bufs=6) as alp,
            tc.tile_pool(name="cast", bufs=3) as scp,
            tc.tile_pool(name="ps_o", bufs=2, space="PSUM") as ps_o,
            tc.tile_pool(name="ps_s", bufs=3, space="PSUM") as ps_s,
        ):
            # The ACT activation-table load costs 1.3us and the framework
            # inserts it before ACT's first instruction: give ACT a no-dep
            # dummy op up front so the load overlaps the input DMA window
            # instead of blocking the first xtT copy. (tanh/identity/copy
            # share one table set, so it loads exactly once.)
            warm = cp.tile([1, 8], BF16)
            nc.gpsimd.memset(warm[:], 0.0)
            nc.scalar.activation(warm[:], warm[:], AF.Tanh)
            # PE p-state warm-up: the PE clock ramps mid -> full only after
            # several us of accumulated busy time, and the serial prologue
            # chain (proj -> xC -> scores) otherwise runs ~2x slow (512-col
            # matmuls measure 0.58us vs 0.26us steady-state). Real-sized
            # (K=128, 512-col) dummy matmuls during the DMA window bank
            # that busy time before proj0 arrives; tiny K=1 dummies do NOT
            # trigger the ramp.
            warmbig = cp.tile([P, 512], BF16)
            nc.gpsimd.memset(warmbig[:], 0.0)
            wp = ps_s.tile([8, 512], F32, tag="s", name="warm_ps")
            for _ in range(13):
                nc.tensor.matmul(
                    wp[:], warmbig[:, :8], warmbig[:], start=True, stop=True,
                    skip_group_check=True,
                )

            # xT chunk0 heads the serial critical chain (proj -> xC ->
            # scores -> mult -> tanh): issue it FIRST on the sync queue,
            # split by contraction half so proj's c0 matmuls can start
            # ~0.4us before c1 lands. DMA issues serialize at ~0.65us each
            # per sequencer; bias/ident ride the gpsimd SWDGE queue so adj0
            # gets the third sync slot.
            # per-chunk xT tiles: proj(nq) depends only on its own chunk's
            # DMA, so no false tile-level coupling to the later bulk loads
            xT_sb = [
                cp.tile([P, IC, 512], BF16, name=f"xT_{nq}")
                for nq in range(N // 512)
            ]
            xT_src = xT_d[:].rearrange("(c p) n -> p c n", p=P)
            nc.sync.dma_start(xT_sb[0][:], xT_src[:, :, bass.ts(0, 512)])
            Wall_sb = cp.tile([P, IC * NH * ATN + ATN], BF16)
            nc.sync.dma_start(Wall_sb[:], W_d[:])
            # bias rides sync slot 3: on the gpsimd SWDGE queue it would
            # queue behind three 384KB xT bulk transfers and gate the first
            # xtT copy by ~2us
            b_sb = cp.tile([P, 1], F32)
            nc.sync.dma_start(b_sb[:], b_d[:])
            for nq in range(1, N // 512):
                nc.gpsimd.dma_start(
                    xT_sb[nq][:], xT_src[:, :, bass.ts(nq, 512)]
                )
            ident = cp.tile([P, P], BF16)
            nc.gpsimd.dma_start(ident[:], id_d[:])
            W_sb = Wall_sb[:, : IC * NH * ATN].rearrange(
                "p (c h o) -> p c h o", c=IC, h=NH
            )
            CT_sb = Wall_sb[:, IC * NH * ATN :]

            xtT = cp.tile([P, N], BF16)
            xCT = cp.tile([P, N], BF16)
            xt4 = cp.tile([P, MC, P], BF16)
            out_sb = cp.tile([P, N], BF16)

            # --- prologue pieces ---
            def emit_xtT(nq):
                # xtT[32h+o, n] = sum_i W[h,i,o] x[n,i] + b[h,o].
                # c-outer/h-inner so the 4 col-groups run concurrently in
                # the PE array; start=True on the first c-chunk per group
                # (has_written clear is per-partition-slice). The bias
                # rides on the ACT copy out of psum.
                pt = ps_s.tile([P, 1024], F32, tag="s", name=f"pj_{nq}")
                for c in range(IC):
                    for h in range(NH):
                        nc.tensor.matmul(
                            pt[bass.ts(h, ATN), :512],
                            W_sb[:, c, h, :],
                            xT_sb[nq][:, c, :],
                            start=(c == 0),
                            stop=(c == IC - 1),
                            tile_position=(0, h * ATN),
                            skip_group_check=True,
                        )
                nc.scalar.activation(
                    xtT[:, bass.ts(nq, 512)], pt[:, :512], AF.Identity, bias=b_sb[:]
                )

            def emit_xCT(nq, on_act=True):
                # xCT[32h+p, n] = sum_o C[h,p,o] xt[n,o]; diagonal 32x32
                # tiles run concurrently in distinct row+col groups.
                pt = ps_s.tile([P, 1024], F32, tag="s", name=f"xc_{nq}")
                for h in range(NH):
                    nc.tensor.matmul(
                        pt[bass.ts(h, ATN), :512],
                        CT_sb[bass.ts(h, ATN), :],
                        xtT[bass.ts(h, ATN), bass.ts(nq, 512)],
                        start=True,
                        stop=True,
                        tile_position=(h * ATN, h * ATN),
                        skip_group_check=True,
                    )
                if on_act:
                    nc.scalar.copy(xCT[:, bass.ts(nq, 512)], pt[:, :512])
                else:
                    nc.vector.tensor_copy(xCT[:, bass.ts(nq, 512)], pt[:, :512])

            def emit_xt4(g):
                # xt4[m_local, mc, f] = xt[mc*128+m_local, f]: PE transposes
                # of xtT, 4 m-chunks per psum tile (cycled through a ps_s
                # slot). Copies on DVE (bf16 psum -> bf16 sbuf, 2x path).
                pt = ps_s.tile([P, 4, P], BF16, tag="s", name=f"tr_{g}")
                for k in range(4):
                    nc.tensor.transpose(
                        pt[:, k, :], xtT[:, bass.ts(4 * g + k, P)], ident[:]
                    )
                nc.vector.tensor_copy(xt4[:, bass.ds(4 * g, 4), :], pt[:])

            # Only projection chunk 0 gates mc0's q0 scores; chunk 1 is
            # emitted between mc0's q-blocks (PROLOG_MID), and chunks 2,3
            # plus the transpose groups are spread through nh0's early
            # m-chunks where PE has slack. heads(mc) first fires at
            # iteration mc+2, so transpose group g is due by mc 4g.
            emit_xtT(0)
            emit_xCT(0)

            def _mid0():
                emit_xtT(1)
                emit_xCT(1)

            # Deferred prologue work rides the pool-routed m-chunks (DVE
            # does one less multiply there, so its copy slots in free);
            # transpose group g is due by iteration 4g+2, proj/xC chunk
            # k by the first scores m-chunk that reads xtT chunk k.
            PROLOG_MID = {0: _mid0}
            PROLOG_AT = {0: lambda: emit_xt4(0),
                         1: lambda: emit_xtT(2),
                         2: lambda: emit_xCT(2, on_act=False),
                         3: lambda: emit_xt4(1),
                         5: lambda: emit_xtT(3),
                         7: lambda: emit_xCT(3, on_act=False),
                         9: lambda: emit_xt4(2),
                         11: lambda: emit_xt4(3)}

            # --- main loop: n-half outer, m-chunks inner, tanh lagging the
            # multiplies by one m-chunk ---
            NHALF = N // 1024
            for nh in range(NHALF):
                # Two 1-bank accumulators; the q0 output copy fires as soon
                # as q0's last matmul stops, overlapping q1's.
                po_q = [
                    ps_o.tile([P, 512], F32, tag="po", name=f"po_{nh}_{q}")
                    for q in range(2)
                ]

                def emit_heads(mc, alpha, q, nh=nh):
                    for h in range(NH):
                        nc.tensor.matmul(
                            po_q[q][bass.ts(h, ATN), :],
                            xt4[:, mc, bass.ts(h, ATN)],
                            alpha[:, h, bass.ts(q, 512)],
                            start=(mc == 0),
                            stop=(mc == MC - 1),
                            tile_position=(0, h * ATN),
                            skip_group_check=True,
                        )

                zbuf = [None, None]
                abuf = [None, None]
                pool_defer = []

                def emit_tanh(mc, nh=nh):
                    alpha = alp.tile([P, NH, 1024], BF16, tag="alpha",
                                     name=f"al_{nh}_{mc}")
                    abuf[mc % 2] = alpha
                    nc.scalar.activation(alpha[:], zbuf[mc % 2][:], AF.Tanh)

                for mc in range(MC):
                    adjt = adjp.tile([P, 1024], BF16, tag="adj")
                    nc.sync.dma_start(
                        adjt[:], adjT_d[bass.ts(mc, P), bass.ds(nh * 1024, 1024)]
                    )
                    zb = zp.tile([P, NH, 1024], BF16, tag="z",
                                 name=f"zb_{nh}_{mc}")
                    zbuf[mc % 2] = zb
                    pool_mc = mc in POOL_MCS
                    # heads(mc-2) interleaved BETWEEN scores groups: its tanh
                    # landed a full m-chunk ago so it never parks waiting
                    # instrs in front of the scores stream (the 4-deep PE
                    # wait queue would head-of-line block DVE's multiply
                    # feed), and splitting the 8-matmul heads burst in two
                    # keeps the 3-slot scores window from draining.
                    # NOTE: the pool-routed slice must be produced FIRST.
                    # Producing it LAST serializes the kernel on an
                    # ACT->GPSIMD->ACT chain (+2.2us per pool mc, measured
                    # 194us); even SECOND measures ~+20us - any
                    # displacement perturbs the 3-slot ps_s rotation more
                    # than the ~4us cast slot-hold costs.
                    qhps = [(q, hp) for q in range(2) for hp in range(NH // 2)]
                    q_seen = set()
                    for q, hp in qhps:
                        if q not in q_seen:
                            q_seen.add(q)
                            if nh == 0 and q == 1 and mc in PROLOG_MID:
                                PROLOG_MID[mc]()
                            if mc >= 2:
                                emit_heads(mc - 2, abuf[mc % 2], q)
                        s2 = ps_s.tile([P, 2, 512], F32, tag="s")
                        for j in range(2):
                            h = 2 * hp + j
                            nc.tensor.matmul(
                                s2[:, j, :],
                                xtT[bass.ts(h, ATN), bass.ts(mc, P)],
                                xCT[
                                    bass.ts(h, ATN),
                                    bass.ds(nh * 1024 + q * 512, 512),
                                ],
                                start=True,
                                stop=True,
                                tile_position=(h * ATN, 0),
                                skip_group_check=True,
                            )
                        zsl = zb[:, bass.ds(2 * hp, 2), bass.ts(q, 512)]
                        adj_b = adjt[:, None, bass.ts(q, 512)].to_broadcast(
                            (P, 2, 512)
                        )
                        if pool_mc and q == 0 and hp == 0:
                            # deferred: ACT casts psum->bf16 after
                            # tanh(mc-1); the otherwise-idle GPSIMD
                            # engine does the multiply (it has no PSUM
                            # port, hence the cast).
                            pool_defer.append((s2, zsl, adj_b))
                        else:
                            nc.vector.tensor_tensor(zsl, s2[:], adj_b, ALU.mult)
                    if nh == 0 and mc in PROLOG_AT:
                        PROLOG_AT[mc]()
                    if mc >= 1:
                        emit_tanh(mc - 1)
                    for (s2, zsl, adj_b) in pool_defer:
                        sc = scp.tile([P, 2, 512], BF16, tag="cast")
                        nc.scalar.copy(sc[:], s2[:])
                        nc.gpsimd.tensor_tensor(zsl, sc[:], adj_b, ALU.mult)
                    pool_defer.clear()
                # tail: heads(14), then per-q tanh(15) + heads(15) + output
                # copy + DMA so the last 512-col DMA chases the last matmul
                for q in range(2):
                    emit_heads(MC - 2, abuf[(MC - 2) % 2], q)
                alpha = alp.tile([P, NH, 1024], BF16, tag="alpha",
                                 name=f"al_{nh}_{MC - 1}")
                for q in range(2):
                    nc.scalar.activation(
                        alpha[:, :, bass.ts(q, 512)],
                        zbuf[(MC - 1) % 2][:, :, bass.ts(q, 512)],
                        AF.Tanh,
                    )
                    emit_heads(MC - 1, alpha, q)
                    if nh == NHALF - 1 and q == 1:
                        # very last output block: split the copy across
                        # ACT+DVE halves with two chasing DMAs to shorten
                        # the serial tail
                        for half, eng in ((0, nc.scalar.copy),
                                          (1, nc.vector.tensor_copy)):
                            off = nh * 1024 + q * 512 + half * 256
                            eng(out_sb[:, bass.ds(off, 256)],
                                po_q[q][:, bass.ts(half, 256)])
                            nc.sync.dma_start(
                                out_d[:, bass.ds(off, 256)],
                                out_sb[:, bass.ds(off, 256)],
                            )
                    else:
                        nc.vector.tensor_copy(
                            out_sb[:, bass.ds(nh * 1024 + q * 512, 512)],
                            po_q[q][:],
                        )
                        nc.sync.dma_start(
                            out_d[:, bass.ds(nh * 1024 + q * 512, 512)],
                            out_sb[:, bass.ds(nh * 1024 + q * 512, 512)],
                        )

    nc.compile()
    return nc


def _get_graph():
    if "nc" not in _CACHE:
        _CACHE["nc"] = build_graph()
    return _CACHE["nc"]


def make_in_maps(x, adj, W, b, C):
    in_maps = []
    for core in range(NCORES):
        bb = core // 2
        hg = core % 2
        hs = slice(hg * NH, (hg + 1) * NH)
        Wt = (
            W[hs]
            .reshape(NH, IC, P, ATN)
            .transpose(2, 1, 0, 3)
            .reshape(P, IC * NH * ATN)
        )
        CTt = C[hs].transpose(0, 2, 1).reshape(NH * ATN, ATN)
        in_maps.append(
            {
                "xT": np.ascontiguousarray(x[bb].T).astype(BF16_NP),
                "ident": np.eye(P, dtype=np.float32).astype(BF16_NP),
                "adjT": np.ascontiguousarray(adj[bb].T).astype(BF16_NP),
                "Wt": np.ascontiguousarray(
                    np.concatenate([Wt, CTt], axis=1)
                ).astype(BF16_NP),
                "bias": np.ascontiguousarray(b[hs].reshape(P, 1)),
            }
        )
    return in_maps


LAST_RESULT = None


def kernel(x, adj, W, b, C):
    global LAST_RESULT
    x = np.asarray(x, dtype=np.float32)
    adj = np.asarray(adj, dtype=np.float32)
    W = np.asarray(W, dtype=np.float32)
    b = np.asarray(b, dtype=np.float32)
    C = np.asarray(C, dtype=np.float32)

    nc = _get_graph()
    in_maps = make_in_maps(x, adj, W, b, C)
    res = run_bass_kernel_spmd(nc, in_maps, core_ids=list(range(NCORES)))
    LAST_RESULT = res

    out = np.empty((B, N, H * ATN), dtype=np.float32)
    for core in range(NCORES):
        bb = core // 2
        hg = core % 2
        out[bb, :, hg * P : (hg + 1) * P] = (
            res.results[core]["out"].astype(np.float32).T
        )
    return out


# revision 67
# speedup vs baseline: 1.0147x; 1.0050x over previous
"""Trainium2 Bass kernel for per-head bilinear graph attention.

Reference computation (B=4, N=2048, IN=256, H=8, ATN=32):
    xt     = einsum('bni,hio->bhno', x, W) + b          # [B,H,N,32]
    xC     = einsum('bhno,hpo->bhnp', xt, C)            # [B,H,N,32]
    scores = einsum('bhnp,bhmp->bhnm', xC, xt)          # [B,H,N,N]
    alpha  = tanh(scores * adj[:,None])                 # [B,H,N,N]
    heads  = einsum('bhnm,bhmo->bhno', alpha, xt)       # [B,H,N,32]
    out    = concat heads on feature dim                # [B,N,256]

Sharding: 8 cores = 4 batches x 2 head-groups (4 heads each). Fully
data-parallel, no collectives. Each core computes out[b, :, hg*128:(hg+1)*128]
transposed ([128, 2048]); the host transposes back and concatenates.

Device-side layout is fully transposed ("T" = [feature/m, n]):
    xtT  [128(4h x 32o), 2048n]   stacked per-head xt^T (bias included)
    xCT  [128(4h x 32p), 2048n]   stacked per-head xC^T
    sT   [128m, n]     = scores[n, m]   (psum, per m-chunk per head)
    z    = sT * adjT   (adjT host-pretransposed so it is [m, n])
    alphaT = tanh(z)
    outT [128(4h x 32o), 2048n] accumulated in psum over 16 m-chunks

Engine budget per core: the kernel sits at the DVE+ACT joint capacity
floor (TRN2 matmul psum output is fp32-only, so every score element must
exit PSUM through a 1x fp32 DVE/ACT op; tanh is ACT-only at 0.83ns/col).
The LP optimum routes 14 of the 128 multiply units through an ACT cast +
GPSIMD multiply and fuses the rest with the psum exit on DVE: ACT ~=
137.8us, DVE ~= 135.5us busy; measured exec 167.6-170.6us at nominal
clock, median ~168.8 (vs 179.5 for the paired-tanh v1 at the same
clock; runs draw a +-20% clock lottery, compare via exec / TANH-busy).
Remaining overhead is ~6.7us fixed NEFF launch, ~8us serial prologue
chain (DMA -> proj -> xC -> scores, latency-tight at each hop), ~5us
tail, ~12us three-engine phase jitter. The scheduling rules that
matter, each worth us-level regressions when violated:
 - per-mc z/alpha tiles [P,4,1024] with tanh pipelined ONE m-chunk
   behind the multiplies (tanh(mc-1) after mc's scores+mults emission),
   deep rotations (z 7, alpha 6, adj 8) so phase jitter never couples
   the engines.
 - heads(mc-2) matmuls interleave BETWEEN the two scores q-groups: a
   contiguous 8-matmul heads burst drains the 3-slot scores window and
   starves DVE ~1.2us per 2 mc; parking heads in front of scores
   head-of-line blocks PE's 4-deep wait queue on ACT's tanh.
 - pool casts are emitted AFTER tanh(mc-1) so ACT never queues a cast
   whose PE slot isn't written yet ahead of a ready tanh; the GPSIMD
   multiply still has ~5us of margin before tanh(mc) needs its slice.
 - no zero-seed matmuls: has_written clears are per-partition-slice on
   this HW (the v1 diagonal xCT matmuls with start=True already relied
   on that), so heads/proj groups use start=(first k-chunk).
 - prologue: a no-dep dummy ACT op hoists the 1.3us activation-table
   load into the DMA window (it otherwise lands between proj0 and the
   first xtT copy); 13 real-sized (K=128, 512-col) dummy matmuls bank
   PE busy time so the mid->full p-state ramp happens inside the DMA
   window instead of stretching the proj/xC/scores chain 2x (tiny K=1
   dummies do NOT trigger the ramp). xT is loaded into per-chunk tiles
   so proj0 depends only on its own DMA; sync queue order xT0, W, bias,
   then adj (bias on the gpsimd SWDGE queue queues behind the 384KB xT
   bulk transfers and gates the first xtT copy ~2us). Only proj/xC
   chunk 0 gates mc0's q0 scores; chunk 1 lands between mc0's q-blocks,
   chunks 2,3 + the xt4 transpose groups ride pool-routed m-chunks
   where DVE has slack.
 - tail: mc15's tanh+heads+output copy are emitted per-q, and the very
   last 512-col block splits its copy across ACT+DVE halves with two
   chasing DMAs.

Key facts baked into this design (from HW traces + the CoreSim cost
model source):
 - DVE tensor_tensor 2x mode needs ALL operands 2-byte; fp32 psum scores
   therefore pin the mask-multiply at 1x. TRN2 matmul cannot write bf16
   psum (is_transpose=True really is a different PE datapath - measured
   garbage), and DMA has no PSUM port, so there is no cheaper exit.
 - GPSIMD cannot access PSUM; its multiplies need an ACT cast first
   (1.0us/unit ACT buys 1.15us/unit off DVE - profitable only while
   ACT's tanh load leaves it slack).
 - K=32 scores matmuls are packed pairwise into PE row-groups
   (tile_position) with [128,2,512] psum tiles; outT uses 4-way
   col-group packing. PSUM: 3x2-bank scores slots + 2x1-bank output
   accumulators = 8 banks.
"""

import sys
import types

import numpy as np
import ml_dtypes

BF16_NP = ml_dtypes.bfloat16


def _ensure_axon_ntff_hook():
    """Provide antenv.axon_hooks if the image lacks it, so
    run_bass_kernel_spmd(trace=True) can capture NTFF profiles instead of
    crashing on the import. No-op when the real module exists."""
    try:
        import antenv.axon_hooks  # noqa: F401

        return
    except ImportError:
        pass
    mod = types.ModuleType("antenv.axon_hooks")
    _state = {"hook": None}
    mod.set_axon_ntff_profile_hook = lambda h: _state.__setitem__("hook", h)
    mod.get_axon_ntff_profile_hook = lambda: _state["hook"]
    sys.modules["antenv.axon_hooks"] = mod
    try:
        import antenv

        antenv.axon_hooks = mod
    except ImportError:
        pass
    try:
        from trn_agent_boot.trn_boot import _ntff_profile_via_ctypes

        mod.set_axon_ntff_profile_hook(
            _ntff_profile_via_ctypes("/opt/axon/libaxon_pjrt.so")
        )
    except Exception:
        pass


_ensure_axon_ntff_hook()

from concourse import bacc, mybir, tile
import concourse.bass as bass
from concourse.bass_utils import run_bass_kernel_spmd

F32 = mybir.dt.float32
BF16 = mybir.dt.bfloat16
AF = mybir.ActivationFunctionType
ALU = mybir.AluOpType

P = 128
B, N, IN_DIM, H, ATN = 4, 2048, 256, 8, 32
NH = 4                # heads per core
NCORES = 8
MC = N // P           # 16 m-chunks
IC = IN_DIM // P      # 2 contraction chunks for the input projection

# m-chunks whose (hp=0, q=0) multiply slice is routed ACT-cast -> GPSIMD
# (per n-half). 7 per half is the measured optimum (g=12/16/18 and the
# even-mc set all cost 1-8us). The cast is emitted right after
# tanh(mc-1): the Pool multiply still has ~5us of margin before tanh(mc)
# needs its z slice, and ACT never queues a cast whose PE slot isn't
# ready ahead of a tanh that is.
POOL_MCS = (1, 3, 5, 7, 9, 11, 13)

_CACHE = {}


def build_graph():
    nc = bacc.Bacc("TRN2", target_bir_lowering=False, debug=False)

    xT_d = nc.dram_tensor("xT", [IN_DIM, N], BF16, kind="ExternalInput")
    id_d = nc.dram_tensor("ident", [P, P], BF16, kind="ExternalInput")
    adjT_d = nc.dram_tensor("adjT", [N, N], BF16, kind="ExternalInput")
    # weights: [P, IC*NH*ATN] W-part ++ [P, ATN] C^T-part, one fast DMA
    W_d = nc.dram_tensor("Wt", [P, IC * NH * ATN + ATN], BF16, kind="ExternalInput")
    b_d = nc.dram_tensor("bias", [P, 1], F32, kind="ExternalInput")
    out_d = nc.dram_tensor("out", [P, N], BF16, kind="ExternalOutput")

    with tile.TileContext(nc) as tc:
        with (
            tc.tile_pool(name="const", bufs=1) as cp,
            tc.tile_pool(name="adj", bufs=8) as adjp,
            tc.tile_pool(name="z", bufs=7) as zp,
            tc.tile_pool(name="alpha", bufs=6) as alp,
            tc.tile_pool(name="cast", bufs=3) as scp,
            tc.tile_pool(name="ps_o", bufs=2, space="PSUM") as ps_o,
            tc.tile_pool(name="ps_s", bufs=3, space="PSUM") as ps_s,
        ):
            # The ACT activation-table load costs 1.3us and the framework
            # inserts it before ACT's first instruction: give ACT a no-dep
            # dummy op up front so the load overlaps the input DMA window
            # instead of blocking the first xtT copy. (tanh/identity/copy
            # share one table set, so it loads exactly once.)
            warm = cp.tile([1, 8], BF16)
            nc.gpsimd.memset(warm[:], 0.0)
            nc.scalar.activation(warm[:], warm[:], AF.Tanh)
            # PE p-state warm-up: the PE clock ramps mid -> full only after
            # several us of accumulated busy time, and the serial prologue
            # chain (proj -> xC -> scores) otherwise runs ~2x slow (512-col
            # matmuls measure 0.58us vs 0.26us steady-state). Real-sized
            # (K=128, 512-col) dummy matmuls during the DMA window bank
            # that busy time before proj0 arrives; tiny K=1 dummies do NOT
            # trigger the ramp.
            warmbig = cp.tile([P, 512], BF16)
            nc.gpsimd.memset(warmbig[:], 0.0)
            wp = ps_s.tile([8, 512], F32, tag="s", name="warm_ps")
            for _ in range(13):
                nc.tensor.matmul(
                    wp[:], warmbig[:, :8], warmbig[:], start=True, stop=True,
                    skip_group_check=True,
                )

            # xT chunk0 heads the serial critical chain (proj -> xC ->
            # scores -> mult -> tanh): issue it FIRST on the sync queue,
            # split by contraction half so proj's c0 matmuls can start
            # ~0.4us before c1 lands. DMA issues serialize at ~0.65us each
            # per sequencer; bias/ident ride the gpsimd SWDGE queue so adj0
            # gets the third sync slot.
            # per-chunk xT tiles: proj(nq) depends only on its own chunk's
            # DMA, so no false tile-level coupling to the later bulk loads
            xT_sb = [
                cp.tile([P, IC, 512], BF16, name=f"xT_{nq}")
                for nq in range(N // 512)
            ]
            xT_src = xT_d[:].rearrange("(c p) n -> p c n", p=P)
            nc.sync.dma_start(xT_sb[0][:], xT_src[:, :, bass.ts(0, 512)])
            # W issues from the ACT HWDGE queue IN PARALLEL with xT0 on
            # sync: serial issue slots cost 0.65us each, and W's landing
            # otherwise co-gates proj0
            Wall_sb = cp.tile([P, IC * NH * ATN + ATN], BF16)
            nc.scalar.dma_start(Wall_sb[:], W_d[:])
            # bias rides sync slot 3: on the gpsimd SWDGE queue it would
            # queue behind three 384KB xT bulk transfers and gate the first
            # xtT copy by ~2us
            b_sb = cp.tile([P, 1], F32)
            nc.sync.dma_start(b_sb[:], b_d[:])
            for nq in range(1, N // 512):
                nc.gpsimd.dma_start(
                    xT_sb[nq][:], xT_src[:, :, bass.ts(nq, 512)]
                )
            ident = cp.tile([P, P], BF16)
            nc.gpsimd.dma_start(ident[:], id_d[:])
            W_sb = Wall_sb[:, : IC * NH * ATN].rearrange(
                "p (c h o) -> p c h o", c=IC, h=NH
            )
            CT_sb = Wall_sb[:, IC * NH * ATN :]

            xtT = cp.tile([P, N], BF16)
            xCT = cp.tile([P, N], BF16)
            xt4 = cp.tile([P, MC, P], BF16)
            out_sb = cp.tile([P, N], BF16)

            # --- prologue pieces ---
            def emit_xtT(nq):
                # xtT[32h+o, n] = sum_i W[h,i,o] x[n,i] + b[h,o].
                # c-outer/h-inner so the 4 col-groups run concurrently in
                # the PE array; start=True on the first c-chunk per group
                # (has_written clear is per-partition-slice). The bias
                # rides on the ACT copy out of psum.
                pt = ps_s.tile([P, 1024], F32, tag="s", name=f"pj_{nq}")
                for c in range(IC):
                    for h in range(NH):
                        nc.tensor.matmul(
                            pt[bass.ts(h, ATN), :512],
                            W_sb[:, c, h, :],
                            xT_sb[nq][:, c, :],
                            start=(c == 0),
                            stop=(c == IC - 1),
                            tile_position=(0, h * ATN),
                            skip_group_check=True,
                        )
                nc.scalar.activation(
                    xtT[:, bass.ts(nq, 512)], pt[:, :512], AF.Identity, bias=b_sb[:]
                )

            def emit_xCT(nq, on_act=True):
                # xCT[32h+p, n] = sum_o C[h,p,o] xt[n,o]; diagonal 32x32
                # tiles run concurrently in distinct row+col groups.
                pt = ps_s.tile([P, 1024], F32, tag="s", name=f"xc_{nq}")
                for h in range(NH):
                    nc.tensor.matmul(
                        pt[bass.ts(h, ATN), :512],
                        CT_sb[bass.ts(h, ATN), :],
                        xtT[bass.ts(h, ATN), bass.ts(nq, 512)],
                        start=True,
                        stop=True,
                        tile_position=(h * ATN, h * ATN),
                        skip_group_check=True,
                    )
                if on_act:
                    nc.scalar.copy(xCT[:, bass.ts(nq, 512)], pt[:, :512])
                else:
                    nc.vector.tensor_copy(xCT[:, bass.ts(nq, 512)], pt[:, :512])

            def emit_xt4(g):
                # xt4[m_local, mc, f] = xt[mc*128+m_local, f]: PE transposes
                # of xtT, 4 m-chunks per psum tile (cycled through a ps_s
                # slot). Copies on DVE (bf16 psum -> bf16 sbuf, 2x path).
                pt = ps_s.tile([P, 4, P], BF16, tag="s", name=f"tr_{g}")
                for k in range(4):
                    nc.tensor.transpose(
                        pt[:, k, :], xtT[:, bass.ts(4 * g + k, P)], ident[:]
                    )
                nc.vector.tensor_copy(xt4[:, bass.ds(4 * g, 4), :], pt[:])

            # Only projection chunk 0 gates mc0's q0 scores; chunk 1 is
            # emitted between mc0's q-blocks (PROLOG_MID), and chunks 2,3
            # plus the transpose groups are spread through nh0's early
            # m-chunks where PE has slack. heads(mc) first fires at
            # iteration mc+2, so transpose group g is due by mc 4g.
            emit_xtT(0)
            emit_xCT(0)

            def _mid0():
                emit_xtT(1)
                emit_xCT(1)

            # Deferred prologue work rides the pool-routed m-chunks (DVE
            # does one less multiply there, so its copy slots in free);
            # transpose group g is due by iteration 4g+2, proj/xC chunk
            # k by the first scores m-chunk that reads xtT chunk k.
            PROLOG_MID = {0: _mid0}
            PROLOG_AT = {0: lambda: emit_xt4(0),
                         1: lambda: emit_xtT(2),
                         2: lambda: emit_xCT(2, on_act=False),
                         3: lambda: emit_xt4(1),
                         5: lambda: emit_xtT(3),
                         7: lambda: emit_xCT(3, on_act=False),
                         9: lambda: emit_xt4(2),
                         11: lambda: emit_xt4(3)}

            # --- main loop: n-half outer, m-chunks inner, tanh lagging the
            # multiplies by one m-chunk ---
            NHALF = N // 1024
            for nh in range(NHALF):
                # Two 1-bank accumulators; the q0 output copy fires as soon
                # as q0's last matmul stops, overlapping q1's.
                po_q = [
                    ps_o.tile([P, 512], F32, tag="po", name=f"po_{nh}_{q}")
                    for q in range(2)
                ]

                def emit_heads(mc, alpha, q, nh=nh):
                    for h in range(NH):
                        nc.tensor.matmul(
                            po_q[q][bass.ts(h, ATN), :],
                            xt4[:, mc, bass.ts(h, ATN)],
                            alpha[:, h, bass.ts(q, 512)],
                            start=(mc == 0),
                            stop=(mc == MC - 1),
                            tile_position=(0, h * ATN),
                            skip_group_check=True,
                        )

                zbuf = [None, None]
                abuf = [None, None]
                pool_defer = []

                def emit_tanh(mc, nh=nh):
                    alpha = alp.tile([P, NH, 1024], BF16, tag="alpha",
                                     name=f"al_{nh}_{mc}")
                    abuf[mc % 2] = alpha
                    nc.scalar.activation(alpha[:], zbuf[mc % 2][:], AF.Tanh)

                for mc in range(MC):
                    adjt = adjp.tile([P, 1024], BF16, tag="adj")
                    nc.sync.dma_start(
                        adjt[:], adjT_d[bass.ts(mc, P), bass.ds(nh * 1024, 1024)]
                    )
                    zb = zp.tile([P, NH, 1024], BF16, tag="z",
                                 name=f"zb_{nh}_{mc}")
                    zbuf[mc % 2] = zb
                    pool_mc = mc in POOL_MCS
                    # heads(mc-2) interleaved BETWEEN scores groups: its tanh
                    # landed a full m-chunk ago so it never parks waiting
                    # instrs in front of the scores stream (the 4-deep PE
                    # wait queue would head-of-line block DVE's multiply
                    # feed), and splitting the 8-matmul heads burst in two
                    # keeps the 3-slot scores window from draining.
                    # NOTE: the pool-routed slice must be produced FIRST.
                    # Producing it LAST serializes the kernel on an
                    # ACT->GPSIMD->ACT chain (+2.2us per pool mc, measured
                    # 194us); even SECOND measures ~+20us - any
                    # displacement perturbs the 3-slot ps_s rotation more
                    # than the ~4us cast slot-hold costs.
                    qhps = [(q, hp) for q in range(2) for hp in range(NH // 2)]
                    q_seen = set()
                    for q, hp in qhps:
                        if q not in q_seen:
                            q_seen.add(q)
                            if nh == 0 and q == 1 and mc in PROLOG_MID:
                                PROLOG_MID[mc]()
                            if mc >= 2:
                                emit_heads(mc - 2, abuf[mc % 2], q)
                        s2 = ps_s.tile([P, 2, 512], F32, tag="s")
                        for j in range(2):
                            h = 2 * hp + j
                            nc.tensor.matmul(
                                s2[:, j, :],
                                xtT[bass.ts(h, ATN), bass.ts(mc, P)],
                                xCT[
                                    bass.ts(h, ATN),
                                    bass.ds(nh * 1024 + q * 512, 512),
                                ],
                                start=True,
                                stop=True,
                                tile_position=(h * ATN, 0),
                                skip_group_check=True,
                            )
                        zsl = zb[:, bass.ds(2 * hp, 2), bass.ts(q, 512)]
                        adj_b = adjt[:, None, bass.ts(q, 512)].to_broadcast(
                            (P, 2, 512)
                        )
                        if pool_mc and q == 0 and hp == 0:
                            # deferred: ACT casts psum->bf16 after
                            # tanh(mc-1); the otherwise-idle GPSIMD
                            # engine does the multiply (it has no PSUM
                            # port, hence the cast).
                            pool_defer.append((s2, zsl, adj_b))
                        else:
                            nc.vector.tensor_tensor(zsl, s2[:], adj_b, ALU.mult)
                    if nh == 0 and mc in PROLOG_AT:
                        PROLOG_AT[mc]()
                    if mc >= 1:
                        emit_tanh(mc - 1)
                    for (s2, zsl, adj_b) in pool_defer:
                        sc = scp.tile([P, 2, 512], BF16, tag="cast")
                        nc.scalar.copy(sc[:], s2[:])
                        nc.gpsimd.tensor_tensor(zsl, sc[:], adj_b, ALU.mult)
                    pool_defer.clear()
                # tail: heads(14), then per-q tanh(15) + heads(15) + output
                # copy + DMA so the last 512-col DMA chases the last matmul
                for q in range(2):
                    emit_heads(MC - 2, abuf[(MC - 2) % 2], q)
                alpha = alp.tile([P, NH, 1024], BF16, tag="alpha",
                                 name=f"al_{nh}_{MC - 1}")
                for q in range(2):
                    nc.scalar.activation(
                        alpha[:, :, bass.ts(q, 512)],
                        zbuf[(MC - 1) % 2][:, :, bass.ts(q, 512)],
                        AF.Tanh,
                    )
                    emit_heads(MC - 1, alpha, q)
                    if nh == NHALF - 1 and q == 1:
                        # very last output block: split the copy across
                        # ACT+DVE halves with two chasing DMAs to shorten
                        # the serial tail
                        for half, eng in ((0, nc.scalar.copy),
                                          (1, nc.vector.tensor_copy)):
                            off = nh * 1024 + q * 512 + half * 256
                            eng(out_sb[:, bass.ds(off, 256)],
                                po_q[q][:, bass.ts(half, 256)])
                            nc.sync.dma_start(
                                out_d[:, bass.ds(off, 256)],
                                out_sb[:, bass.ds(off, 256)],
                            )
                    else:
                        nc.vector.tensor_copy(
                            out_sb[:, bass.ds(nh * 1024 + q * 512, 512)],
                            po_q[q][:],
                        )
                        nc.sync.dma_start(
                            out_d[:, bass.ds(nh * 1024 + q * 512, 512)],
                            out_sb[:, bass.ds(nh * 1024 + q * 512, 512)],
                        )

    nc.compile()
    return nc


def _get_graph():
    if "nc" not in _CACHE:
        _CACHE["nc"] = build_graph()
    return _CACHE["nc"]


def make_in_maps(x, adj, W, b, C):
    in_maps = []
    for core in range(NCORES):
        bb = core // 2
        hg = core % 2
        hs = slice(hg * NH, (hg + 1) * NH)
        Wt = (
            W[hs]
            .reshape(NH, IC, P, ATN)
            .transpose(2, 1, 0, 3)
            .reshape(P, IC * NH * ATN)
        )
        CTt = C[hs].transpose(0, 2, 1).reshape(NH * ATN, ATN)
        in_maps.append(
            {
                "xT": np.ascontiguousarray(x[bb].T).astype(BF16_NP),
                "ident": np.eye(P, dtype=np.float32).astype(BF16_NP),
                "adjT": np.ascontiguousarray(adj[bb].T).astype(BF16_NP),
                "Wt": np.ascontiguousarray(
                    np.concatenate([Wt, CTt], axis=1)
                ).astype(BF16_NP),
                "bias": np.ascontiguousarray(b[hs].reshape(P, 1)),
            }
        )
    return in_maps


LAST_RESULT = None


def kernel(x, adj, W, b, C):
    global LAST_RESULT
    x = np.asarray(x, dtype=np.float32)
    adj = np.asarray(adj, dtype=np.float32)
    W = np.asarray(W, dtype=np.float32)
    b = np.asarray(b, dtype=np.float32)
    C = np.asarray(C, dtype=np.float32)

    nc = _get_graph()
    in_maps = make_in_maps(x, adj, W, b, C)
    res = run_bass_kernel_spmd(nc, in_maps, core_ids=list(range(NCORES)))
    LAST_RESULT = res

    out = np.empty((B, N, H * ATN), dtype=np.float32)
    for core in range(NCORES):
        bb = core // 2
        hg = core % 2
        out[bb, :, hg * P : (hg + 1) * P] = (
            res.results[core]["out"].astype(np.float32).T
        )
    return out
